# revision 20
# baseline (speedup 1.0000x reference)
"""Causal self-attention (B=4, T=2048, C=1024, H=16) on 8 trn2 NeuronCores.

Sharding: core c -> batch b = c//2, heads h0 = (c%2)*8 .. h0+8 (tensor
parallel over heads: c_attn columns / c_proj rows split). Each core computes a
partial projection output [T, C]; the host sums the two partials per batch and
adds b_proj.

Device-side dataflow (all matmuls in float32r = full PE rate, fp32 data):
  - host passes x[b] pre-transposed as xt [C, T]
  - qT, kT  [C_head, T] computed with W_attn column-slices as stationary
  - v computed in natural [T, D] layout, augmented with a ones column so the
    PV matmul also produces the softmax denominator (row 64 of yT_aug)
  - S^T tiles [Tk=128, Tq<=512] = kT_tile^T . qT_chunk  (causal: only the
    lower triangle of S, i.e. Tq >= Tk tiles, is computed)
  - P~ = exp(S^T * 0.125) on ScalarE (no max-subtraction: scores are O(1));
    diagonal 128x128 blocks masked with an upper-triangular 0/1 mask
  - yT_aug [65, T] += v_aug_tile^T . P~  accumulated in PSUM over k-tiles
  - normalize: reciprocal of row 64, gpsimd partition-broadcast, DVE multiply
  - proj: out_tile [128, C] = yT_tile^T . W_proj_rows, streamed to DRAM
"""

import numpy as np

P = 128


def build_program(T=2048, C=1024, HC=8, D=64, num_devices=8, trn="TRN2"):
    import concourse.mybir as mybir
    import concourse.tile as tile
    from concourse import bacc
    from concourse.masks import make_upper_triangular

    W = min(512, T)  # matmul moving-dim chunk
    WS = min(1024, T)  # score-PSUM superchunk (exp granularity)
    KC = C // P      # contraction tiles over C
    CO = HC * D      # this core's qkv channel block (512)
    NP = CO // P     # head pairs (2 heads of 64 = 1 partition tile)
    TT = T // P      # T tiles
    NCH = T // W     # T chunks
    WO = min(512, C)  # proj output column chunk
    NW = C // WO     # output column chunks
    dt32 = mybir.dt.float32
    f32r = mybir.dt.float32r
    ActF = mybir.ActivationFunctionType
    Alu = mybir.AluOpType
    scale = 1.0 / float(np.sqrt(D))

    nc = bacc.Bacc(trn, target_bir_lowering=False, debug=False,
                   enable_asserts=False, num_devices=num_devices)

    xt_d = nc.dram_tensor("xt", [C, T], f32r, kind="ExternalInput")
    wq_d = nc.dram_tensor("wq", [C, CO], f32r, kind="ExternalInput")
    wk_d = nc.dram_tensor("wk", [C, CO], f32r, kind="ExternalInput")
    wv_d = nc.dram_tensor("wv", [C, CO], f32r, kind="ExternalInput")
    bq_d = nc.dram_tensor("bq", [P, NP], dt32, kind="ExternalInput")
    bk_d = nc.dram_tensor("bk", [P, NP], dt32, kind="ExternalInput")
    bvb_d = nc.dram_tensor("bvb", [P, CO], dt32, kind="ExternalInput")
    ones_d = nc.dram_tensor("ones", [P, TT * HC], f32r, kind="ExternalInput")
    wp_d = nc.dram_tensor("wp", [CO, C], f32r, kind="ExternalInput")
    out_d = nc.dram_tensor("out", [T, C], dt32, kind="ExternalOutput")

    with tile.TileContext(nc) as tc:
        with tc.tile_pool(name="const", bufs=1) as cpool, \
             tc.tile_pool(name="pers", bufs=1) as pers:
            tri = cpool.tile([P, P], dt32)
            make_upper_triangular(nc, tri[:], val=1.0, diag=True)
            bq_sb = cpool.tile([P, NP], dt32)
            nc.sync.dma_start(bq_sb[:], bq_d.ap())
            bk_sb = cpool.tile([P, NP], dt32)
            nc.sync.dma_start(bk_sb[:], bk_d.ap())
            bvb_sb = cpool.tile([P, CO], dt32)
            nc.sync.dma_start(bvb_sb[:], bvb_d.ap())

            qT = pers.tile([P, NP, T], f32r, tag="qT")
            kT = pers.tile([P, NP, T], f32r, tag="kT")
            vaug = pers.tile([P, TT, HC, D + 1], f32r, tag="vaug")
            nc.sync.dma_start(
                vaug[:, :, :, D],
                ones_d.ap().rearrange("p (a b) -> p a b", b=HC))

            # ---------------- stage B: qkv projections ----------------
            with nc.named_scope("qkv"), \
                 tc.tile_pool(name="xtp", bufs=KC) as xpool, \
                 tc.tile_pool(name="wp_in", bufs=1) as wpool, \
                 tc.tile_pool(name="psB", bufs=2, space="PSUM") as psB:
                xt_view = xt_d.ap().rearrange("(kc p) t -> kc p t", p=P)
                xts = []
                for kc in range(KC):
                    xte = xpool.tile([P, T], f32r, tag="xt")
                    nc.sync.dma_start(xte[:], xt_view[kc])
                    xts.append(xte)

                def qk_stage(w_d, dst, bias_sb):
                    wsb = wpool.tile([P, KC, CO], f32r, tag="w")
                    nc.sync.dma_start(
                        wsb[:], w_d.ap().rearrange("(kc p) n -> p kc n", p=P))
                    for m in range(NP):
                        ps = psB.tile([P, T], dt32, tag="psB")
                        for cg in range(NCH):
                            for kc in range(KC):
                                nc.tensor.matmul(
                                    ps[:, cg * W:(cg + 1) * W],
                                    wsb[:, kc, m * P:(m + 1) * P],
                                    xts[kc][:, cg * W:(cg + 1) * W],
                                    start=(kc == 0), stop=(kc == KC - 1),
                                    skip_group_check=True)
                        nc.scalar.activation(
                            dst[:, m, :], ps[:],
                            ActF.Identity, bias=bias_sb[:, m:m + 1], scale=1.0)

                qk_stage(wq_d, qT, bq_sb)
                qk_stage(wk_d, kT, bk_sb)

                wvsb = wpool.tile([P, KC, CO], f32r, tag="w")
                nc.sync.dma_start(
                    wvsb[:], wv_d.ap().rearrange("(kc p) n -> p kc n", p=P))
                bvb_v = bvb_sb[:].rearrange("p (h d) -> p h d", d=D)
                for tt in range(TT):
                    ps = psB.tile([P, CO], dt32, tag="psB")
                    for kc in range(KC):
                        nc.tensor.matmul(
                            ps[:],
                            xts[kc][:, tt * P:(tt + 1) * P],
                            wvsb[:, kc, :],
                            start=(kc == 0), stop=(kc == KC - 1))
                    nc.vector.scalar_tensor_tensor(
                        out=vaug[:, tt, :, 0:D],
                        in0=ps[:].rearrange("p (h d) -> p h d", d=D),
                        scalar=1.0, in1=bvb_v,
                        op0=Alu.mult, op1=Alu.add)

            # ---------------- stage C: attention per head ----------------
            late_cm = tc.tile_pool(name="late", bufs=1)
            late = late_cm.__enter__()
            yT = late.tile([P, NP, T], f32r, tag="yT")
            with nc.named_scope("attn"), \
                 tc.tile_pool(name="ptp", bufs=2) as ptpool, \
                 tc.tile_pool(name="nrm", bufs=2) as nrmpool, \
                 tc.tile_pool(name="psS", bufs=2, space="PSUM") as psS, \
                 tc.tile_pool(name="psY", bufs=1, space="PSUM") as psY:
                for h in range(HC):
                    m, r0 = h // 2, (h % 2) * D
                    yt = psY.tile([D + 1, T], dt32, tag="yt")
                    for j in range(TT):
                        jb = j * P
                        span = T - jb
                        pt = ptpool.tile([P, span], f32r, tag="pt")
                        for sc in range(jb // WS, T // WS):
                            qs0 = max(WS * sc, jb)
                            sps = psS.tile([P, WS], dt32, tag="s")
                            for cg in range(qs0 // W, (WS * (sc + 1)) // W):
                                qs = max(W * cg, qs0)
                                w = W * (cg + 1) - qs
                                nc.tensor.matmul(
                                    sps[:, qs - WS * sc:qs - WS * sc + w],
                                    kT[r0:r0 + D, m, jb:jb + P],
                                    qT[r0:r0 + D, m, qs:qs + w],
                                    start=True, stop=True,
                                    skip_group_check=True)
                            nc.scalar.activation(
                                pt[:, qs0 - jb:WS * (sc + 1) - jb],
                                sps[:, qs0 - WS * sc:WS],
                                ActF.Exp, scale=scale)
                        nc.vector.tensor_mul(pt[:, 0:P], pt[:, 0:P], tri[:])
                        for cg in range(jb // W, NCH):
                            qs = max(W * cg, jb)
                            w = W * (cg + 1) - qs
                            last_j = (W * (cg + 1)) // P - 1
                            nc.tensor.matmul(
                                yt[:, qs:qs + w],
                                vaug[:, j, h, :],
                                pt[:, qs - jb:qs - jb + w],
                                start=(j == 0), stop=(j == last_j),
                                skip_group_check=True)
                    rcp = nrmpool.tile([1, T], dt32, tag="rcp")
                    nc.vector.reciprocal(rcp[:], yt[D:D + 1, :])
                    bc = nrmpool.tile([D, T], dt32, tag="bc")
                    nc.gpsimd.partition_broadcast(bc[:], rcp[:])
                    nc.vector.tensor_mul(yT[r0:r0 + D, m, :], yt[0:D, :], bc[:])

            # ---------------- stage E: output projection ----------------
            with nc.named_scope("proj"), \
                 tc.tile_pool(name="wpp", bufs=1) as wppool, \
                 tc.tile_pool(name="ost", bufs=3) as opool, \
                 tc.tile_pool(name="psO", bufs=2, space="PSUM") as psO:
                wpsb = wppool.tile([P, NP, C], f32r)
                nc.sync.dma_start(
                    wpsb[:], wp_d.ap().rearrange("(kt p) n -> p kt n", p=P))
                for tt in range(TT):
                    po = psO.tile([P, C], dt32, tag="o")
                    for kt in range(NP):
                        for nn in range(NW):
                            nc.tensor.matmul(
                                po[:, nn * WO:(nn + 1) * WO],
                                yT[:, kt, tt * P:(tt + 1) * P],
                                wpsb[:, kt, nn * WO:(nn + 1) * WO],
                                start=(kt == 0), stop=(kt == NP - 1),
                                skip_group_check=True)
                    ot = opool.tile([P, C], dt32, tag="ot")
                    nc.scalar.activation(ot[:], po[:], ActF.Copy)
                    nc.sync.dma_start(out_d.ap()[tt * P:(tt + 1) * P, :], ot[:])
            late_cm.__exit__(None, None, None)

    nc.compile()
    return nc


def make_core_inputs(x, W_attn, b_attn, W_proj, n_cores=8, HC=8, D=64):
    """Host-side sharding: per-core input dicts."""
    B, T, C = x.shape
    CO = HC * D
    NP = CO // P
    in_maps = []
    for c in range(n_cores):
        b = c // (n_cores // B)
        h0 = (c % (n_cores // B)) * HC
        lo = h0 * D
        bq = b_attn[lo:lo + CO]
        bk = b_attn[C + lo:C + lo + CO]
        bv = b_attn[2 * C + lo:2 * C + lo + CO]
        in_maps.append({
            "xt": np.ascontiguousarray(x[b].T),
            "wq": np.ascontiguousarray(W_attn[:, lo:lo + CO]),
            "wk": np.ascontiguousarray(W_attn[:, C + lo:C + lo + CO]),
            "wv": np.ascontiguousarray(W_attn[:, 2 * C + lo:2 * C + lo + CO]),
            "bq": np.ascontiguousarray(bq.reshape(NP, P).T),
            "bk": np.ascontiguousarray(bk.reshape(NP, P).T),
            "bvb": np.tile(bv[None, :], (P, 1)),
            "ones": np.ones((P, (T // P) * HC), np.float32),
            "wp": np.ascontiguousarray(W_proj[lo:lo + CO, :]),
        })
    return in_maps


_CACHE = {}


def _get_program():
    if "nc" not in _CACHE:
        _CACHE["nc"] = build_program()
    return _CACHE["nc"]


def run_on_cores(x, W_attn, b_attn, W_proj, b_proj, trace=False):
    """Returns (full output [B,T,C], BassKernelResults)."""
    from concourse.bass_utils import run_bass_kernel_spmd

    x = np.asarray(x, np.float32)
    W_attn = np.asarray(W_attn, np.float32)
    b_attn = np.asarray(b_attn, np.float32)
    W_proj = np.asarray(W_proj, np.float32)
    b_proj = np.asarray(b_proj, np.float32)

    nc = _get_program()
    in_maps = make_core_inputs(x, W_attn, b_attn, W_proj)
    res = run_bass_kernel_spmd(nc, in_maps, core_ids=list(range(8)), trace=trace)
    B, T, C = x.shape
    out = np.empty((B, T, C), np.float32)
    for b in range(B):
        out[b] = (res.results[2 * b]["out"] + res.results[2 * b + 1]["out"]
                  + b_proj[None, :])
    return out, res


def kernel(x, W_attn, b_attn, W_proj, b_proj):
    out, _ = run_on_cores(x, W_attn, b_attn, W_proj, b_proj, trace=False)
    return out


# revision 26
# speedup vs baseline: 1.0553x; 1.0553x over previous
"""Causal self-attention (B=4, T=2048, C=1024, H=16) on 8 trn2 NeuronCores.

Sharding: core c -> batch b = c//2, heads h0 = (c%2)*8 .. h0+8 (tensor
parallel over heads: c_attn columns / c_proj rows split). Each core computes a
partial projection output [T, C]; the host sums the two partials per batch and
adds b_proj.

Device-side dataflow (all matmuls in float32r = full PE rate, fp32 data):
  - host passes x[b] pre-transposed as xt [C, T]
  - qT, kT  [C_head, T] computed with W_attn column-slices as stationary
  - v computed in natural [T, D] layout, augmented with a ones column so the
    PV matmul also produces the softmax denominator (row 64 of yT_aug)
  - S^T tiles [Tk=128, Tq<=512] = kT_tile^T . qT_chunk  (causal: only the
    lower triangle of S, i.e. Tq >= Tk tiles, is computed)
  - P~ = exp(S^T * 0.125) on ScalarE (no max-subtraction: scores are O(1));
    diagonal 128x128 blocks masked with an upper-triangular 0/1 mask
  - yT_aug [65, T] += v_aug_tile^T . P~  accumulated in PSUM over k-tiles
  - normalize: reciprocal of row 64, gpsimd partition-broadcast, DVE multiply
  - proj: out_tile [128, C] = yT_tile^T . W_proj_rows, streamed to DRAM
"""

import numpy as np

P = 128


def build_program(T=2048, C=1024, HC=8, D=64, num_devices=8, trn="TRN2"):
    import concourse.mybir as mybir
    import concourse.tile as tile
    from concourse import bacc
    from concourse.masks import make_upper_triangular

    W = min(512, T)  # matmul moving-dim chunk
    WS = min(1024, T)  # score-PSUM superchunk (exp granularity)
    KC = C // P      # contraction tiles over C
    CO = HC * D      # this core's qkv channel block (512)
    NP = CO // P     # head pairs (2 heads of 64 = 1 partition tile)
    TT = T // P      # T tiles
    NCH = T // W     # T chunks
    WO = min(512, C)  # proj output column chunk
    NW = C // WO     # output column chunks
    dt32 = mybir.dt.float32
    f32r = mybir.dt.float32r
    ActF = mybir.ActivationFunctionType
    Alu = mybir.AluOpType
    scale = 1.0 / float(np.sqrt(D))

    nc = bacc.Bacc(trn, target_bir_lowering=False, debug=False,
                   enable_asserts=False, num_devices=num_devices)

    xt_d = nc.dram_tensor("xt", [C, T], f32r, kind="ExternalInput")
    wq_d = nc.dram_tensor("wq", [C, CO], f32r, kind="ExternalInput")
    wk_d = nc.dram_tensor("wk", [C, CO], f32r, kind="ExternalInput")
    wv_d = nc.dram_tensor("wv", [C, CO], f32r, kind="ExternalInput")
    bq_d = nc.dram_tensor("bq", [P, NP], dt32, kind="ExternalInput")
    bk_d = nc.dram_tensor("bk", [P, NP], dt32, kind="ExternalInput")
    bvb_d = nc.dram_tensor("bvb", [P, CO], dt32, kind="ExternalInput")
    ones_d = nc.dram_tensor("ones", [P, TT * HC], f32r, kind="ExternalInput")
    wp_d = nc.dram_tensor("wp", [CO, C], f32r, kind="ExternalInput")
    out_d = nc.dram_tensor("out", [T, C], dt32, kind="ExternalOutput")

    with tile.TileContext(nc) as tc:
        with tc.tile_pool(name="const", bufs=1) as cpool, \
             tc.tile_pool(name="pers", bufs=1) as pers:
            tri = cpool.tile([P, P], dt32)
            make_upper_triangular(nc, tri[:], val=1.0, diag=True)
            bq_sb = cpool.tile([P, NP], dt32)
            nc.sync.dma_start(bq_sb[:], bq_d.ap())
            bk_sb = cpool.tile([P, NP], dt32)
            nc.sync.dma_start(bk_sb[:], bk_d.ap())
            bvb_sb = cpool.tile([P, CO], dt32)
            nc.sync.dma_start(bvb_sb[:], bvb_d.ap())

            qT = pers.tile([P, NP, T], f32r, tag="qT")
            kT = pers.tile([P, NP, T], f32r, tag="kT")
            vaug = pers.tile([P, TT, HC, D + 1], f32r, tag="vaug")
            nc.sync.dma_start(
                vaug[:, :, :, D],
                ones_d.ap().rearrange("p (a b) -> p a b", b=HC))

            # ---------------- stage B: qkv projections ----------------
            with nc.named_scope("qkv"), \
                 tc.tile_pool(name="xtp", bufs=KC) as xpool, \
                 tc.tile_pool(name="wp_in", bufs=KC) as wpool, \
                 tc.tile_pool(name="psB", bufs=2, space="PSUM") as psB:
                xt_view = xt_d.ap().rearrange("(kc p) t -> kc p t", p=P)
                wq_view = wq_d.ap().rearrange("(kc p) n -> kc p n", p=P)
                xts = []
                wq_t = []
                for kc in range(KC):
                    xte = xpool.tile([P, T], f32r, tag="xt")
                    nc.sync.dma_start(xte[:], xt_view[kc])
                    xts.append(xte)
                    wt = wpool.tile([P, CO], f32r, tag="w")
                    nc.sync.dma_start(wt[:], wq_view[kc])
                    wq_t.append(wt)

                def qk_stage(w_tiles, dst, bias_sb):
                    for m in range(NP):
                        ps = psB.tile([P, T], dt32, tag="psB")
                        for cg in range(NCH):
                            for kc in range(KC):
                                nc.tensor.matmul(
                                    ps[:, cg * W:(cg + 1) * W],
                                    w_tiles[kc][:, m * P:(m + 1) * P],
                                    xts[kc][:, cg * W:(cg + 1) * W],
                                    start=(kc == 0), stop=(kc == KC - 1),
                                    skip_group_check=True)
                        nc.scalar.activation(
                            dst[:, m, :], ps[:],
                            ActF.Identity, bias=bias_sb[:, m:m + 1], scale=1.0)

                def load_w(w_d):
                    view = w_d.ap().rearrange("(kc p) n -> kc p n", p=P)
                    tiles = []
                    for kc in range(KC):
                        wt = wpool.tile([P, CO], f32r, tag="w")
                        nc.sync.dma_start(wt[:], view[kc])
                        tiles.append(wt)
                    return tiles

                qk_stage(wq_t, qT, bq_sb)
                qk_stage(load_w(wk_d), kT, bk_sb)

                wv_t = load_w(wv_d)
                bvb_v = bvb_sb[:].rearrange("p (h d) -> p h d", d=D)
                for tt in range(TT):
                    ps = psB.tile([P, CO], dt32, tag="psB")
                    for kc in range(KC):
                        nc.tensor.matmul(
                            ps[:],
                            xts[kc][:, tt * P:(tt + 1) * P],
                            wv_t[kc][:],
                            start=(kc == 0), stop=(kc == KC - 1))
                    nc.vector.scalar_tensor_tensor(
                        out=vaug[:, tt, :, 0:D],
                        in0=ps[:].rearrange("p (h d) -> p h d", d=D),
                        scalar=1.0, in1=bvb_v,
                        op0=Alu.mult, op1=Alu.add)

            # ---------------- stage C: attention per head ----------------
            late_cm = tc.tile_pool(name="late", bufs=1)
            late = late_cm.__enter__()
            yT = late.tile([P, NP, T], f32r, tag="yT")
            with nc.named_scope("attn"), \
                 tc.tile_pool(name="ptp", bufs=2) as ptpool, \
                 tc.tile_pool(name="nrm", bufs=1) as nrmpool, \
                 tc.tile_pool(name="ysp", bufs=2) as yspool, \
                 tc.tile_pool(name="psS", bufs=2, space="PSUM") as psS, \
                 tc.tile_pool(name="psY", bufs=1, space="PSUM") as psY:
                for h in range(HC):
                    m, r0 = h // 2, (h % 2) * D
                    yt = psY.tile([D + 1, T], dt32, tag="yt")
                    for j in range(TT):
                        jb = j * P
                        span = T - jb
                        pt = ptpool.tile([P, span], f32r, tag="pt")
                        for sc in range(jb // WS, T // WS):
                            qs0 = max(WS * sc, jb)
                            sps = psS.tile([P, WS], dt32, tag="s")
                            for cg in range(qs0 // W, (WS * (sc + 1)) // W):
                                qs = max(W * cg, qs0)
                                w = W * (cg + 1) - qs
                                nc.tensor.matmul(
                                    sps[:, qs - WS * sc:qs - WS * sc + w],
                                    kT[r0:r0 + D, m, jb:jb + P],
                                    qT[r0:r0 + D, m, qs:qs + w],
                                    start=True, stop=True,
                                    skip_group_check=True)
                            nc.scalar.activation(
                                pt[:, qs0 - jb:WS * (sc + 1) - jb],
                                sps[:, qs0 - WS * sc:WS],
                                ActF.Exp, scale=scale)
                        nc.vector.tensor_mul(pt[:, 0:P], pt[:, 0:P], tri[:])
                        for cg in range(jb // W, NCH):
                            qs = max(W * cg, jb)
                            w = W * (cg + 1) - qs
                            last_j = (W * (cg + 1)) // P - 1
                            nc.tensor.matmul(
                                yt[:, qs:qs + w],
                                vaug[:, j, h, :],
                                pt[:, qs - jb:qs - jb + w],
                                start=(j == 0), stop=(j == last_j),
                                skip_group_check=True)
                    # copy PSUM accumulator out quickly to release it for the
                    # next head; the slow normalize chain then runs SBUF-side.
                    ys = yspool.tile([D + 1, T], dt32, tag="ys")
                    nc.vector.tensor_copy(ys[:], yt[:])
                    rcp = nrmpool.tile([1, T], dt32, tag="rcp")
                    nc.vector.reciprocal(rcp[:], ys[D:D + 1, :])
                    bc = nrmpool.tile([D, T], dt32, tag="bc")
                    nc.gpsimd.partition_broadcast(bc[:], rcp[:])
                    nc.vector.tensor_mul(yT[r0:r0 + D, m, :], ys[0:D, :], bc[:])

            # ---------------- stage E: output projection ----------------
            with nc.named_scope("proj"), \
                 tc.tile_pool(name="wpp", bufs=1) as wppool, \
                 tc.tile_pool(name="ost", bufs=3) as opool, \
                 tc.tile_pool(name="psO", bufs=2, space="PSUM") as psO:
                wpsb = wppool.tile([P, NP, C], f32r)
                nc.sync.dma_start(
                    wpsb[:], wp_d.ap().rearrange("(kt p) n -> p kt n", p=P))
                for tt in range(TT):
                    po = psO.tile([P, C], dt32, tag="o")
                    for kt in range(NP):
                        for nn in range(NW):
                            nc.tensor.matmul(
                                po[:, nn * WO:(nn + 1) * WO],
                                yT[:, kt, tt * P:(tt + 1) * P],
                                wpsb[:, kt, nn * WO:(nn + 1) * WO],
                                start=(kt == 0), stop=(kt == NP - 1),
                                skip_group_check=True)
                    ot = opool.tile([P, C], dt32, tag="ot")
                    nc.scalar.activation(ot[:], po[:], ActF.Copy)
                    nc.sync.dma_start(out_d.ap()[tt * P:(tt + 1) * P, :], ot[:])
            late_cm.__exit__(None, None, None)

    nc.compile()
    return nc


def make_core_inputs(x, W_attn, b_attn, W_proj, n_cores=8, HC=8, D=64):
    """Host-side sharding: per-core input dicts."""
    B, T, C = x.shape
    CO = HC * D
    NP = CO // P
    in_maps = []
    for c in range(n_cores):
        b = c // (n_cores // B)
        h0 = (c % (n_cores // B)) * HC
        lo = h0 * D
        bq = b_attn[lo:lo + CO]
        bk = b_attn[C + lo:C + lo + CO]
        bv = b_attn[2 * C + lo:2 * C + lo + CO]
        in_maps.append({
            "xt": np.ascontiguousarray(x[b].T),
            "wq": np.ascontiguousarray(W_attn[:, lo:lo + CO]),
            "wk": np.ascontiguousarray(W_attn[:, C + lo:C + lo + CO]),
            "wv": np.ascontiguousarray(W_attn[:, 2 * C + lo:2 * C + lo + CO]),
            "bq": np.ascontiguousarray(bq.reshape(NP, P).T),
            "bk": np.ascontiguousarray(bk.reshape(NP, P).T),
            "bvb": np.tile(bv[None, :], (P, 1)),
            "ones": np.ones((P, (T // P) * HC), np.float32),
            "wp": np.ascontiguousarray(W_proj[lo:lo + CO, :]),
        })
    return in_maps


_CACHE = {}


def _get_program():
    if "nc" not in _CACHE:
        _CACHE["nc"] = build_program()
    return _CACHE["nc"]


def run_on_cores(x, W_attn, b_attn, W_proj, b_proj, trace=False):
    """Returns (full output [B,T,C], BassKernelResults)."""
    from concourse.bass_utils import run_bass_kernel_spmd

    x = np.asarray(x, np.float32)
    W_attn = np.asarray(W_attn, np.float32)
    b_attn = np.asarray(b_attn, np.float32)
    W_proj = np.asarray(W_proj, np.float32)
    b_proj = np.asarray(b_proj, np.float32)

    nc = _get_program()
    in_maps = make_core_inputs(x, W_attn, b_attn, W_proj)
    res = run_bass_kernel_spmd(nc, in_maps, core_ids=list(range(8)), trace=trace)
    B, T, C = x.shape
    out = np.empty((B, T, C), np.float32)
    for b in range(B):
        out[b] = (res.results[2 * b]["out"] + res.results[2 * b + 1]["out"]
                  + b_proj[None, :])
    return out, res


def kernel(x, W_attn, b_attn, W_proj, b_proj):
    out, _ = run_on_cores(x, W_attn, b_attn, W_proj, b_proj, trace=False)
    return out


# revision 29
# speedup vs baseline: 1.0852x; 1.0283x over previous
"""Causal self-attention (B=4, T=2048, C=1024, H=16) on 8 trn2 NeuronCores.

Sharding: core c -> batch b = c//2, heads h0 = (c%2)*8 .. h0+8 (tensor
parallel over heads: c_attn columns / c_proj rows split). Each core computes a
partial projection output [T, C]; the host sums the two partials per batch and
adds b_proj.

Device-side dataflow (all matmuls in float32r = full PE rate, fp32 data):
  - host passes x[b] pre-transposed as xt [C, T]
  - qT, kT  [C_head, T] computed with W_attn column-slices as stationary
  - v computed in natural [T, D] layout, augmented with a ones column so the
    PV matmul also produces the softmax denominator (row 64 of yT_aug)
  - S^T tiles [Tk=128, Tq<=512] = kT_tile^T . qT_chunk  (causal: only the
    lower triangle of S, i.e. Tq >= Tk tiles, is computed)
  - P~ = exp(S^T * 0.125) on ScalarE (no max-subtraction: scores are O(1));
    diagonal 128x128 blocks masked with an upper-triangular 0/1 mask
  - yT_aug [65, T] += v_aug_tile^T . P~  accumulated in PSUM over k-tiles
  - normalize: reciprocal of row 64, gpsimd partition-broadcast, DVE multiply
  - proj: out_tile [128, C] = yT_tile^T . W_proj_rows, streamed to DRAM
"""

import numpy as np

P = 128


def _bf16_np():
    import ml_dtypes
    return ml_dtypes.bfloat16


def build_program(T=2048, C=1024, HC=8, D=64, num_devices=8, trn="TRN2"):
    import concourse.mybir as mybir
    import concourse.tile as tile
    from concourse import bacc
    from concourse.masks import make_upper_triangular

    W = min(512, T)  # matmul moving-dim chunk
    WS = min(1024, T)  # score-PSUM superchunk (exp granularity)
    KC = C // P      # contraction tiles over C
    CO = HC * D      # this core's qkv channel block (512)
    NP = CO // P     # head pairs (2 heads of 64 = 1 partition tile)
    TT = T // P      # T tiles
    NCH = T // W     # T chunks
    WO = min(512, C)  # proj output column chunk
    NW = C // WO     # output column chunks
    dt32 = mybir.dt.float32
    f32r = mybir.dt.float32r
    bf16 = mybir.dt.bfloat16
    ActF = mybir.ActivationFunctionType
    Alu = mybir.AluOpType
    scale = 1.0 / float(np.sqrt(D))

    nc = bacc.Bacc(trn, target_bir_lowering=False, debug=False,
                   enable_asserts=False, num_devices=num_devices)

    xt_d = nc.dram_tensor("xt", [C, T], f32r, kind="ExternalInput")
    wq_d = nc.dram_tensor("wq", [C, CO], f32r, kind="ExternalInput")
    wk_d = nc.dram_tensor("wk", [C, CO], f32r, kind="ExternalInput")
    wv_d = nc.dram_tensor("wv", [C, CO], f32r, kind="ExternalInput")
    bq_d = nc.dram_tensor("bq", [P, NP], dt32, kind="ExternalInput")
    bk_d = nc.dram_tensor("bk", [P, NP], dt32, kind="ExternalInput")
    bvb_d = nc.dram_tensor("bvb", [P, CO], dt32, kind="ExternalInput")
    ones_d = nc.dram_tensor("ones", [P, TT * HC], bf16, kind="ExternalInput")
    wp_d = nc.dram_tensor("wp", [CO, C], f32r, kind="ExternalInput")
    out_d = nc.dram_tensor("out", [T, C], dt32, kind="ExternalOutput")

    with tile.TileContext(nc) as tc:
        with tc.tile_pool(name="const", bufs=1) as cpool, \
             tc.tile_pool(name="pers", bufs=1) as pers:
            tri = cpool.tile([P, P], bf16)
            make_upper_triangular(nc, tri[:], val=1.0, diag=True)
            bq_sb = cpool.tile([P, NP], dt32)
            nc.sync.dma_start(bq_sb[:], bq_d.ap())
            bk_sb = cpool.tile([P, NP], dt32)
            nc.sync.dma_start(bk_sb[:], bk_d.ap())
            bvb_sb = cpool.tile([P, CO], dt32)
            nc.sync.dma_start(bvb_sb[:], bvb_d.ap())

            qT = pers.tile([P, NP, T], bf16, tag="qT")
            kT = pers.tile([P, NP, T], bf16, tag="kT")
            vaug = pers.tile([P, TT, HC, D + 1], bf16, tag="vaug")
            nc.sync.dma_start(
                vaug[:, :, :, D],
                ones_d.ap().rearrange("p (a b) -> p a b", b=HC))

            # ---------------- stage B: qkv projections ----------------
            with nc.named_scope("qkv"), \
                 tc.tile_pool(name="xtp", bufs=KC) as xpool, \
                 tc.tile_pool(name="wp_in", bufs=KC) as wpool, \
                 tc.tile_pool(name="psB", bufs=2, space="PSUM") as psB:
                xt_view = xt_d.ap().rearrange("(kc p) t -> kc p t", p=P)
                wq_view = wq_d.ap().rearrange("(kc p) n -> kc p n", p=P)
                xts = []
                wq_t = []
                for kc in range(KC):
                    xte = xpool.tile([P, T], f32r, tag="xt")
                    nc.sync.dma_start(xte[:], xt_view[kc])
                    xts.append(xte)
                    wt = wpool.tile([P, CO], f32r, tag="w")
                    nc.sync.dma_start(wt[:], wq_view[kc])
                    wq_t.append(wt)

                def qk_stage(w_tiles, dst, bias_sb):
                    for m in range(NP):
                        ps = psB.tile([P, T], dt32, tag="psB")
                        for cg in range(NCH):
                            for kc in range(KC):
                                nc.tensor.matmul(
                                    ps[:, cg * W:(cg + 1) * W],
                                    w_tiles[kc][:, m * P:(m + 1) * P],
                                    xts[kc][:, cg * W:(cg + 1) * W],
                                    start=(kc == 0), stop=(kc == KC - 1),
                                    skip_group_check=True)
                        nc.scalar.activation(
                            dst[:, m, :], ps[:],
                            ActF.Identity, bias=bias_sb[:, m:m + 1], scale=1.0)

                def load_w(w_d):
                    view = w_d.ap().rearrange("(kc p) n -> kc p n", p=P)
                    tiles = []
                    for kc in range(KC):
                        wt = wpool.tile([P, CO], f32r, tag="w")
                        nc.sync.dma_start(wt[:], view[kc])
                        tiles.append(wt)
                    return tiles

                qk_stage(wq_t, qT, bq_sb)
                qk_stage(load_w(wk_d), kT, bk_sb)

                wv_t = load_w(wv_d)
                bvb_v = bvb_sb[:].rearrange("p (h d) -> p h d", d=D)
                for tt in range(TT):
                    ps = psB.tile([P, CO], dt32, tag="psB")
                    for kc in range(KC):
                        nc.tensor.matmul(
                            ps[:],
                            xts[kc][:, tt * P:(tt + 1) * P],
                            wv_t[kc][:],
                            start=(kc == 0), stop=(kc == KC - 1))
                    nc.vector.scalar_tensor_tensor(
                        out=vaug[:, tt, :, 0:D],
                        in0=ps[:].rearrange("p (h d) -> p h d", d=D),
                        scalar=1.0, in1=bvb_v,
                        op0=Alu.mult, op1=Alu.add)

            # ---------------- stage C: attention per head ----------------
            late_cm = tc.tile_pool(name="late", bufs=1)
            late = late_cm.__enter__()
            yT = late.tile([P, NP, T], f32r, tag="yT")
            with nc.named_scope("attn"), \
                 tc.tile_pool(name="ptp", bufs=2) as ptpool, \
                 tc.tile_pool(name="nrm", bufs=1) as nrmpool, \
                 tc.tile_pool(name="ysp", bufs=2) as yspool, \
                 tc.tile_pool(name="psS", bufs=2, space="PSUM") as psS, \
                 tc.tile_pool(name="psY", bufs=1, space="PSUM") as psY:
                for h in range(HC):
                    m, r0 = h // 2, (h % 2) * D
                    yt = psY.tile([D + 1, T], dt32, tag="yt")
                    for j in range(TT):
                        jb = j * P
                        span = T - jb
                        pt = ptpool.tile([P, span], bf16, tag="pt")
                        for sc in range(jb // WS, T // WS):
                            qs0 = max(WS * sc, jb)
                            sps = psS.tile([P, WS], dt32, tag="s")
                            for cg in range(qs0 // W, (WS * (sc + 1)) // W):
                                qs = max(W * cg, qs0)
                                w = W * (cg + 1) - qs
                                nc.tensor.matmul(
                                    sps[:, qs - WS * sc:qs - WS * sc + w],
                                    kT[r0:r0 + D, m, jb:jb + P],
                                    qT[r0:r0 + D, m, qs:qs + w],
                                    start=True, stop=True,
                                    skip_group_check=True)
                            nc.scalar.activation(
                                pt[:, qs0 - jb:WS * (sc + 1) - jb],
                                sps[:, qs0 - WS * sc:WS],
                                ActF.Exp, scale=scale)
                        nc.vector.tensor_mul(pt[:, 0:P], pt[:, 0:P], tri[:])
                        for cg in range(jb // W, NCH):
                            qs = max(W * cg, jb)
                            w = W * (cg + 1) - qs
                            last_j = (W * (cg + 1)) // P - 1
                            nc.tensor.matmul(
                                yt[:, qs:qs + w],
                                vaug[:, j, h, :],
                                pt[:, qs - jb:qs - jb + w],
                                start=(j == 0), stop=(j == last_j),
                                skip_group_check=True)
                    # copy PSUM accumulator out quickly to release it for the
                    # next head; the slow normalize chain then runs SBUF-side.
                    ys = yspool.tile([D + 1, T], dt32, tag="ys")
                    nc.scalar.activation(ys[:], yt[:], ActF.Copy)
                    rcp = nrmpool.tile([1, T], dt32, tag="rcp")
                    nc.vector.reciprocal(rcp[:], ys[D:D + 1, :])
                    bc = nrmpool.tile([D, T], dt32, tag="bc")
                    nc.gpsimd.partition_broadcast(bc[:], rcp[:])
                    nc.vector.tensor_mul(yT[r0:r0 + D, m, :], ys[0:D, :], bc[:])

            # ---------------- stage E: output projection ----------------
            with nc.named_scope("proj"), \
                 tc.tile_pool(name="wpp", bufs=1) as wppool, \
                 tc.tile_pool(name="ost", bufs=3) as opool, \
                 tc.tile_pool(name="psO", bufs=2, space="PSUM") as psO:
                wpsb = wppool.tile([P, NP, C], f32r)
                nc.sync.dma_start(
                    wpsb[:], wp_d.ap().rearrange("(kt p) n -> p kt n", p=P))
                for tt in range(TT):
                    po = psO.tile([P, C], dt32, tag="o")
                    for kt in range(NP):
                        for nn in range(NW):
                            nc.tensor.matmul(
                                po[:, nn * WO:(nn + 1) * WO],
                                yT[:, kt, tt * P:(tt + 1) * P],
                                wpsb[:, kt, nn * WO:(nn + 1) * WO],
                                start=(kt == 0), stop=(kt == NP - 1),
                                skip_group_check=True)
                    ot = opool.tile([P, C], dt32, tag="ot")
                    nc.scalar.activation(ot[:], po[:], ActF.Copy)
                    nc.sync.dma_start(out_d.ap()[tt * P:(tt + 1) * P, :], ot[:])
            late_cm.__exit__(None, None, None)

    nc.compile()
    return nc


def make_core_inputs(x, W_attn, b_attn, W_proj, n_cores=8, HC=8, D=64):
    """Host-side sharding: per-core input dicts."""
    B, T, C = x.shape
    CO = HC * D
    NP = CO // P
    in_maps = []
    for c in range(n_cores):
        b = c // (n_cores // B)
        h0 = (c % (n_cores // B)) * HC
        lo = h0 * D
        bq = b_attn[lo:lo + CO]
        bk = b_attn[C + lo:C + lo + CO]
        bv = b_attn[2 * C + lo:2 * C + lo + CO]
        in_maps.append({
            "xt": np.ascontiguousarray(x[b].T),
            "wq": np.ascontiguousarray(W_attn[:, lo:lo + CO]),
            "wk": np.ascontiguousarray(W_attn[:, C + lo:C + lo + CO]),
            "wv": np.ascontiguousarray(W_attn[:, 2 * C + lo:2 * C + lo + CO]),
            "bq": np.ascontiguousarray(bq.reshape(NP, P).T),
            "bk": np.ascontiguousarray(bk.reshape(NP, P).T),
            "bvb": np.tile(bv[None, :], (P, 1)),
            "ones": np.ones((P, (T // P) * HC), _bf16_np()),
            "wp": np.ascontiguousarray(W_proj[lo:lo + CO, :]),
        })
    return in_maps


_CACHE = {}


def _get_program():
    if "nc" not in _CACHE:
        _CACHE["nc"] = build_program()
    return _CACHE["nc"]


def run_on_cores(x, W_attn, b_attn, W_proj, b_proj, trace=False):
    """Returns (full output [B,T,C], BassKernelResults)."""
    from concourse.bass_utils import run_bass_kernel_spmd

    x = np.asarray(x, np.float32)
    W_attn = np.asarray(W_attn, np.float32)
    b_attn = np.asarray(b_attn, np.float32)
    W_proj = np.asarray(W_proj, np.float32)
    b_proj = np.asarray(b_proj, np.float32)

    nc = _get_program()
    in_maps = make_core_inputs(x, W_attn, b_attn, W_proj)
    res = run_bass_kernel_spmd(nc, in_maps, core_ids=list(range(8)), trace=trace)
    B, T, C = x.shape
    out = np.empty((B, T, C), np.float32)
    for b in range(B):
        out[b] = (res.results[2 * b]["out"] + res.results[2 * b + 1]["out"]
                  + b_proj[None, :])
    return out, res


def kernel(x, W_attn, b_attn, W_proj, b_proj):
    out, _ = run_on_cores(x, W_attn, b_attn, W_proj, b_proj, trace=False)
    return out


# revision 31
# speedup vs baseline: 1.4137x; 1.3028x over previous
"""Causal self-attention (B=4, T=2048, C=1024, H=16) on 8 trn2 NeuronCores.

Sharding: core c -> batch b = c//2, heads h0 = (c%2)*8 .. h0+8 (tensor
parallel over heads: c_attn columns / c_proj rows split). Each core computes a
partial projection output [T, C]; the host sums the two partials per batch and
adds b_proj.

Device-side dataflow (all matmuls in float32r = full PE rate, fp32 data):
  - host passes x[b] pre-transposed as xt [C, T]
  - qT, kT  [C_head, T] computed with W_attn column-slices as stationary
  - v computed in natural [T, D] layout, augmented with a ones column so the
    PV matmul also produces the softmax denominator (row 64 of yT_aug)
  - S^T tiles [Tk=128, Tq<=512] = kT_tile^T . qT_chunk  (causal: only the
    lower triangle of S, i.e. Tq >= Tk tiles, is computed)
  - P~ = exp(S^T * 0.125) on ScalarE (no max-subtraction: scores are O(1));
    diagonal 128x128 blocks masked with an upper-triangular 0/1 mask
  - yT_aug [65, T] += v_aug_tile^T . P~  accumulated in PSUM over k-tiles
  - normalize: reciprocal of row 64, gpsimd partition-broadcast, DVE multiply
  - proj: out_tile [128, C] = yT_tile^T . W_proj_rows, streamed to DRAM
"""

import numpy as np

P = 128


def _bf16_np():
    import ml_dtypes
    return ml_dtypes.bfloat16


def build_program(T=2048, C=1024, HC=8, D=64, num_devices=8, trn="TRN2"):
    import concourse.mybir as mybir
    import concourse.tile as tile
    from concourse import bacc
    from concourse.masks import make_upper_triangular

    W = min(512, T)  # matmul moving-dim chunk
    WS = min(1024, T)  # score-PSUM superchunk (exp granularity)
    KC = C // P      # contraction tiles over C
    CO = HC * D      # this core's qkv channel block (512)
    NP = CO // P     # head pairs (2 heads of 64 = 1 partition tile)
    TT = T // P      # T tiles
    NCH = T // W     # T chunks
    WO = min(512, C)  # proj output column chunk
    NW = C // WO     # output column chunks
    dt32 = mybir.dt.float32
    f32r = mybir.dt.float32r
    bf16 = mybir.dt.bfloat16
    ActF = mybir.ActivationFunctionType
    Alu = mybir.AluOpType
    scale = 1.0 / float(np.sqrt(D))

    nc = bacc.Bacc(trn, target_bir_lowering=False, debug=False,
                   enable_asserts=False, num_devices=num_devices)

    xt_d = nc.dram_tensor("xt", [C, T], f32r, kind="ExternalInput")
    wq_d = nc.dram_tensor("wq", [C, CO], f32r, kind="ExternalInput")
    wk_d = nc.dram_tensor("wk", [C, CO], f32r, kind="ExternalInput")
    wv_d = nc.dram_tensor("wv", [C, CO], f32r, kind="ExternalInput")
    bq_d = nc.dram_tensor("bq", [P, NP], dt32, kind="ExternalInput")
    bk_d = nc.dram_tensor("bk", [P, NP], dt32, kind="ExternalInput")
    bvb_d = nc.dram_tensor("bvb", [P, CO], dt32, kind="ExternalInput")
    ones_d = nc.dram_tensor("ones", [P, TT * HC], bf16, kind="ExternalInput")
    wp_d = nc.dram_tensor("wp", [CO, C], f32r, kind="ExternalInput")
    out_d = nc.dram_tensor("out", [T, C], dt32, kind="ExternalOutput")
    lsc_d = nc.dram_tensor("lsc", [T], dt32)
    lsc2_d = nc.dram_tensor("lsc2", [T], dt32)

    with tile.TileContext(nc) as tc:
        with tc.tile_pool(name="const", bufs=1) as cpool, \
             tc.tile_pool(name="pers", bufs=1) as pers:
            tri = cpool.tile([P, P], bf16)
            make_upper_triangular(nc, tri[:], val=1.0, diag=True)
            bq_sb = cpool.tile([P, NP], dt32)
            nc.sync.dma_start(bq_sb[:], bq_d.ap())
            bk_sb = cpool.tile([P, NP], dt32)
            nc.sync.dma_start(bk_sb[:], bk_d.ap())
            bvb_sb = cpool.tile([P, CO], dt32)
            nc.sync.dma_start(bvb_sb[:], bvb_d.ap())

            MV = 96  # PV stationary columns (3 full 32-col PE groups)
            qT = pers.tile([P, NP, T], bf16, tag="qT")
            kT = pers.tile([P, HC, T], bf16, tag="kT")
            vaug = pers.tile([P, TT, HC, MV], bf16, tag="vaug")
            nc.gpsimd.memset(kT[:], 0.0)
            nc.gpsimd.memset(vaug[:], 0.0)
            nc.sync.dma_start(
                vaug[:, :, :, D],
                ones_d.ap().rearrange("p (a b) -> p a b", b=HC))

            # ---------------- stage B: qkv projections ----------------
            with nc.named_scope("qkv"), \
                 tc.tile_pool(name="xtp", bufs=KC) as xpool, \
                 tc.tile_pool(name="wp_in", bufs=KC) as wpool, \
                 tc.tile_pool(name="psB", bufs=2, space="PSUM") as psB:
                xt_view = xt_d.ap().rearrange("(kc p) t -> kc p t", p=P)
                wq_view = wq_d.ap().rearrange("(kc p) n -> kc p n", p=P)
                xts = []
                wq_t = []
                dmae = [nc.sync, nc.scalar, nc.gpsimd]
                for kc in range(KC):
                    xte = xpool.tile([P, T], f32r, tag="xt")
                    dmae[kc % 3].dma_start(xte[:], xt_view[kc])
                    xts.append(xte)
                    wt = wpool.tile([P, CO], f32r, tag="w")
                    dmae[(kc + 1) % 3].dma_start(wt[:], wq_view[kc])
                    wq_t.append(wt)

                def qk_stage(w_tiles, bias_sb, write_out):
                    for m in range(NP):
                        ps = psB.tile([P, T], dt32, tag="psB")
                        for cg in range(NCH):
                            for kc in range(KC):
                                nc.tensor.matmul(
                                    ps[:, cg * W:(cg + 1) * W],
                                    w_tiles[kc][:, m * P:(m + 1) * P],
                                    xts[kc][:, cg * W:(cg + 1) * W],
                                    start=(kc == 0), stop=(kc == KC - 1),
                                    skip_group_check=True)
                        write_out(m, ps)

                def write_qT(m, ps):
                    nc.scalar.activation(
                        qT[:, m, :], ps[:],
                        ActF.Identity, bias=bq_sb[:, m:m + 1], scale=1.0)

                def write_kT(m, ps):
                    # unpacked per-head, sibling rows stay zero
                    nc.scalar.activation(
                        kT[0:D, 2 * m, :], ps[0:D, :],
                        ActF.Identity, bias=bk_sb[0:D, m:m + 1], scale=1.0)
                    nc.scalar.activation(
                        kT[D:P, 2 * m + 1, :], ps[D:P, :],
                        ActF.Identity, bias=bk_sb[D:P, m:m + 1], scale=1.0)

                def load_w(w_d):
                    view = w_d.ap().rearrange("(kc p) n -> kc p n", p=P)
                    tiles = []
                    for kc in range(KC):
                        wt = wpool.tile([P, CO], f32r, tag="w")
                        [nc.sync, nc.scalar, nc.gpsimd][kc % 3].dma_start(
                            wt[:], view[kc])
                        tiles.append(wt)
                    return tiles

                qk_stage(wq_t, bq_sb, write_qT)
                qk_stage(load_w(wk_d), bk_sb, write_kT)

                wv_t = load_w(wv_d)
                bvb_v = bvb_sb[:].rearrange("p (h d) -> p h d", d=D)
                for tt in range(TT):
                    ps = psB.tile([P, CO], dt32, tag="psB")
                    for kc in range(KC):
                        nc.tensor.matmul(
                            ps[:],
                            xts[kc][:, tt * P:(tt + 1) * P],
                            wv_t[kc][:],
                            start=(kc == 0), stop=(kc == KC - 1))
                    nc.vector.scalar_tensor_tensor(
                        out=vaug[:, tt, :, 0:D],
                        in0=ps[:].rearrange("p (h d) -> p h d", d=D),
                        scalar=1.0, in1=bvb_v,
                        op0=Alu.mult, op1=Alu.add)

            # ---------------- stage C: attention per head ----------------
            late_cm = tc.tile_pool(name="late", bufs=1)
            late = late_cm.__enter__()
            yT = late.tile([P, NP, T], f32r, tag="yT")
            with nc.named_scope("attn"), \
                 tc.tile_pool(name="ptp", bufs=2) as ptpool, \
                 tc.tile_pool(name="nrm", bufs=1) as nrmpool, \
                 tc.tile_pool(name="ysp", bufs=2) as yspool, \
                 tc.tile_pool(name="psS", bufs=2, space="PSUM") as psS, \
                 tc.tile_pool(name="psY", bufs=1, space="PSUM") as psY:
                for h in range(HC):
                    m, r0 = h // 2, (h % 2) * D
                    yt = psY.tile([MV, T], dt32, tag="yt")
                    for j in range(TT):
                        jb = j * P
                        span = T - jb
                        pt = ptpool.tile([P, span], bf16, tag="pt")
                        for sc in range(jb // WS, T // WS):
                            qs0 = max(WS * sc, jb)
                            sps = psS.tile([P, WS], dt32, tag="s")
                            for cg in range(qs0 // W, (WS * (sc + 1)) // W):
                                qs = max(W * cg, qs0)
                                w = W * (cg + 1) - qs
                                nc.tensor.matmul(
                                    sps[:, qs - WS * sc:qs - WS * sc + w],
                                    kT[:, h, jb:jb + P],
                                    qT[:, m, qs:qs + w],
                                    start=True, stop=True,
                                    skip_group_check=True)
                            nc.scalar.activation(
                                pt[:, qs0 - jb:WS * (sc + 1) - jb],
                                sps[:, qs0 - WS * sc:WS],
                                ActF.Exp, scale=scale)
                        nc.vector.tensor_mul(pt[:, 0:P], pt[:, 0:P], tri[:])
                        for cg in range(jb // W, NCH):
                            qs = max(W * cg, jb)
                            w = W * (cg + 1) - qs
                            last_j = (W * (cg + 1)) // P - 1
                            nc.tensor.matmul(
                                yt[:, qs:qs + w],
                                vaug[:, j, h, :],
                                pt[:, qs - jb:qs - jb + w],
                                start=(j == 0), stop=(j == last_j),
                                skip_group_check=True)
                    # copy PSUM accumulator out quickly to release it for the
                    # next head; the slow normalize chain then runs SBUF-side.
                    ys = yspool.tile([D + 1, T], dt32, tag="ys")
                    nc.scalar.activation(ys[:], yt[0:D + 1, :], ActF.Copy)
                    nc.sync.dma_start(
                        lsc_d.ap().rearrange("(o t) -> o t", o=1),
                        ys[D:D + 1, :])
                    l128 = nrmpool.tile([P, T // P], dt32, tag="l128")
                    nc.gpsimd.dma_start(
                        l128[:], lsc_d.ap().rearrange("(p c) -> p c", p=P))
                    nc.vector.reciprocal(l128[:], l128[:])
                    nc.scalar.dma_start(
                        lsc2_d.ap().rearrange("(p c) -> p c", p=P), l128[:])
                    rcp = nrmpool.tile([1, T], dt32, tag="rcp")
                    nc.sync.dma_start(
                        rcp[:], lsc2_d.ap().rearrange("(o t) -> o t", o=1))
                    bc = nrmpool.tile([D, T], dt32, tag="bc")
                    nc.gpsimd.partition_broadcast(bc[:], rcp[:])
                    nc.vector.tensor_mul(yT[r0:r0 + D, m, :], ys[0:D, :], bc[:])

            # ---------------- stage E: output projection ----------------
            with nc.named_scope("proj"), \
                 tc.tile_pool(name="wpp", bufs=1) as wppool, \
                 tc.tile_pool(name="ost", bufs=3) as opool, \
                 tc.tile_pool(name="psO", bufs=2, space="PSUM") as psO:
                wpsb = wppool.tile([P, NP, C], f32r)
                nc.sync.dma_start(
                    wpsb[:], wp_d.ap().rearrange("(kt p) n -> p kt n", p=P))
                for tt in range(TT):
                    po = psO.tile([P, C], dt32, tag="o")
                    for kt in range(NP):
                        for nn in range(NW):
                            nc.tensor.matmul(
                                po[:, nn * WO:(nn + 1) * WO],
                                yT[:, kt, tt * P:(tt + 1) * P],
                                wpsb[:, kt, nn * WO:(nn + 1) * WO],
                                start=(kt == 0), stop=(kt == NP - 1),
                                skip_group_check=True)
                    ot = opool.tile([P, C], dt32, tag="ot")
                    nc.scalar.activation(ot[:], po[:], ActF.Copy)
                    nc.sync.dma_start(out_d.ap()[tt * P:(tt + 1) * P, :], ot[:])
            late_cm.__exit__(None, None, None)

    nc.compile()
    return nc


def make_core_inputs(x, W_attn, b_attn, W_proj, n_cores=8, HC=8, D=64):
    """Host-side sharding: per-core input dicts."""
    B, T, C = x.shape
    CO = HC * D
    NP = CO // P
    in_maps = []
    for c in range(n_cores):
        b = c // (n_cores // B)
        h0 = (c % (n_cores // B)) * HC
        lo = h0 * D
        bq = b_attn[lo:lo + CO]
        bk = b_attn[C + lo:C + lo + CO]
        bv = b_attn[2 * C + lo:2 * C + lo + CO]
        in_maps.append({
            "xt": np.ascontiguousarray(x[b].T),
            "wq": np.ascontiguousarray(W_attn[:, lo:lo + CO]),
            "wk": np.ascontiguousarray(W_attn[:, C + lo:C + lo + CO]),
            "wv": np.ascontiguousarray(W_attn[:, 2 * C + lo:2 * C + lo + CO]),
            "bq": np.ascontiguousarray(bq.reshape(NP, P).T),
            "bk": np.ascontiguousarray(bk.reshape(NP, P).T),
            "bvb": np.tile(bv[None, :], (P, 1)),
            "ones": np.ones((P, (T // P) * HC), _bf16_np()),
            "wp": np.ascontiguousarray(W_proj[lo:lo + CO, :]),
        })
    return in_maps


_CACHE = {}


def _get_program():
    if "nc" not in _CACHE:
        _CACHE["nc"] = build_program()
    return _CACHE["nc"]


def run_on_cores(x, W_attn, b_attn, W_proj, b_proj, trace=False):
    """Returns (full output [B,T,C], BassKernelResults)."""
    from concourse.bass_utils import run_bass_kernel_spmd

    x = np.asarray(x, np.float32)
    W_attn = np.asarray(W_attn, np.float32)
    b_attn = np.asarray(b_attn, np.float32)
    W_proj = np.asarray(W_proj, np.float32)
    b_proj = np.asarray(b_proj, np.float32)

    nc = _get_program()
    in_maps = make_core_inputs(x, W_attn, b_attn, W_proj)
    res = run_bass_kernel_spmd(nc, in_maps, core_ids=list(range(8)), trace=trace)
    B, T, C = x.shape
    out = np.empty((B, T, C), np.float32)
    for b in range(B):
        out[b] = (res.results[2 * b]["out"] + res.results[2 * b + 1]["out"]
                  + b_proj[None, :])
    return out, res


def kernel(x, W_attn, b_attn, W_proj, b_proj):
    out, _ = run_on_cores(x, W_attn, b_attn, W_proj, b_proj, trace=False)
    return out


# revision 33
# speedup vs baseline: 1.4223x; 1.0061x over previous
"""Causal self-attention (B=4, T=2048, C=1024, H=16) on 8 trn2 NeuronCores.

Sharding: core c -> batch b = c//2, heads h0 = (c%2)*8 .. h0+8 (tensor
parallel over heads: c_attn columns / c_proj rows split). Each core computes a
partial projection output [T, C]; the host sums the two partials per batch and
adds b_proj.

Device-side dataflow (all matmuls in float32r = full PE rate, fp32 data):
  - host passes x[b] pre-transposed as xt [C, T]
  - qT, kT  [C_head, T] computed with W_attn column-slices as stationary
  - v computed in natural [T, D] layout, augmented with a ones column so the
    PV matmul also produces the softmax denominator (row 64 of yT_aug)
  - S^T tiles [Tk=128, Tq<=512] = kT_tile^T . qT_chunk  (causal: only the
    lower triangle of S, i.e. Tq >= Tk tiles, is computed)
  - P~ = exp(S^T * 0.125) on ScalarE (no max-subtraction: scores are O(1));
    diagonal 128x128 blocks masked with an upper-triangular 0/1 mask
  - yT_aug [65, T] += v_aug_tile^T . P~  accumulated in PSUM over k-tiles
  - normalize: reciprocal of row 64, gpsimd partition-broadcast, DVE multiply
  - proj: out_tile [128, C] = yT_tile^T . W_proj_rows, streamed to DRAM
"""

import numpy as np

P = 128


def _bf16_np():
    import ml_dtypes
    return ml_dtypes.bfloat16


def build_program(T=2048, C=1024, HC=8, D=64, num_devices=8, trn="TRN2"):
    import concourse.mybir as mybir
    import concourse.tile as tile
    from concourse import bacc
    from concourse.masks import make_upper_triangular

    W = min(512, T)  # matmul moving-dim chunk
    WS = min(1024, T)  # score-PSUM superchunk (exp granularity)
    KC = C // P      # contraction tiles over C
    CO = HC * D      # this core's qkv channel block (512)
    NP = CO // P     # head pairs (2 heads of 64 = 1 partition tile)
    TT = T // P      # T tiles
    NCH = T // W     # T chunks
    WO = min(512, C)  # proj output column chunk
    NW = C // WO     # output column chunks
    dt32 = mybir.dt.float32
    f32r = mybir.dt.float32r
    bf16 = mybir.dt.bfloat16
    ActF = mybir.ActivationFunctionType
    Alu = mybir.AluOpType
    scale = 1.0 / float(np.sqrt(D))

    nc = bacc.Bacc(trn, target_bir_lowering=False, debug=False,
                   enable_asserts=False, num_devices=num_devices)

    xt_d = nc.dram_tensor("xt", [C, T], f32r, kind="ExternalInput")
    wq_d = nc.dram_tensor("wq", [C, CO], f32r, kind="ExternalInput")
    wk_d = nc.dram_tensor("wk", [C, CO], f32r, kind="ExternalInput")
    wv_d = nc.dram_tensor("wv", [C, CO], f32r, kind="ExternalInput")
    bq_d = nc.dram_tensor("bq", [P, NP], dt32, kind="ExternalInput")
    bk_d = nc.dram_tensor("bk", [P, NP], dt32, kind="ExternalInput")
    bvb_d = nc.dram_tensor("bvb", [P, CO], dt32, kind="ExternalInput")
    ones_d = nc.dram_tensor("ones", [P, TT * HC], bf16, kind="ExternalInput")
    wp_d = nc.dram_tensor("wp", [CO, C], f32r, kind="ExternalInput")
    out_d = nc.dram_tensor("out", [T, C], dt32, kind="ExternalOutput")
    lsc_d = nc.dram_tensor("lsc", [T], dt32)
    lsc2_d = nc.dram_tensor("lsc2", [T], dt32)

    with tile.TileContext(nc) as tc:
        with tc.tile_pool(name="const", bufs=1) as cpool, \
             tc.tile_pool(name="pers", bufs=1) as pers:
            tri = cpool.tile([P, P], bf16)
            make_upper_triangular(nc, tri[:], val=1.0, diag=True)
            bq_sb = cpool.tile([P, NP], dt32)
            nc.sync.dma_start(bq_sb[:], bq_d.ap())
            bk_sb = cpool.tile([P, NP], dt32)
            nc.sync.dma_start(bk_sb[:], bk_d.ap())
            bvb_sb = cpool.tile([P, CO], dt32)
            nc.sync.dma_start(bvb_sb[:], bvb_d.ap())

            MV = 96  # PV stationary columns (3 full 32-col PE groups)
            qT = pers.tile([P, NP, T], bf16, tag="qT")
            kT = pers.tile([P, HC, T], bf16, tag="kT")
            vaug = pers.tile([P, TT, HC, MV], bf16, tag="vaug")
            nc.gpsimd.memset(kT[:], 0.0)
            nc.gpsimd.memset(vaug[:], 0.0)
            nc.sync.dma_start(
                vaug[:, :, :, D],
                ones_d.ap().rearrange("p (a b) -> p a b", b=HC))

            # ---------------- stage B: qkv projections ----------------
            with nc.named_scope("qkv"), \
                 tc.tile_pool(name="xtp", bufs=KC) as xpool, \
                 tc.tile_pool(name="wp_in", bufs=KC) as wpool, \
                 tc.tile_pool(name="psB", bufs=2, space="PSUM") as psB:
                xt_view = xt_d.ap().rearrange("(kc p) t -> kc p t", p=P)
                wq_view = wq_d.ap().rearrange("(kc p) n -> kc p n", p=P)
                xts = []
                wq_t = []
                dmae = [nc.sync, nc.scalar, nc.gpsimd]
                for kc in range(KC):
                    xte = xpool.tile([P, T], f32r, tag="xt")
                    dmae[kc % 3].dma_start(xte[:], xt_view[kc])
                    xts.append(xte)
                    wt = wpool.tile([P, CO], f32r, tag="w")
                    dmae[(kc + 1) % 3].dma_start(wt[:], wq_view[kc])
                    wq_t.append(wt)

                def qk_stage(w_tiles, bias_sb, write_out):
                    for half in range((NP + 1) // 2):
                        ms = [m for m in (2 * half, 2 * half + 1) if m < NP]
                        pss = {}
                        for m in ms:
                            ps_m = psB.tile([P, T], dt32, tag="psB")
                            pss[m] = ps_m
                        for kc in range(KC):
                            for m in ms:
                                for cg in range(NCH):
                                    nc.tensor.matmul(
                                        pss[m][:, cg * W:(cg + 1) * W],
                                        w_tiles[kc][:, m * P:(m + 1) * P],
                                        xts[kc][:, cg * W:(cg + 1) * W],
                                        start=(kc == 0), stop=(kc == KC - 1),
                                        skip_group_check=True)
                        for m in ms:
                            write_out(m, pss[m])

                def write_qT(m, ps):
                    nc.scalar.activation(
                        qT[:, m, :], ps[:],
                        ActF.Identity, bias=bq_sb[:, m:m + 1], scale=1.0)

                def write_kT(m, ps):
                    # unpacked per-head, sibling rows stay zero
                    nc.scalar.activation(
                        kT[0:D, 2 * m, :], ps[0:D, :],
                        ActF.Identity, bias=bk_sb[0:D, m:m + 1], scale=1.0)
                    nc.scalar.activation(
                        kT[D:P, 2 * m + 1, :], ps[D:P, :],
                        ActF.Identity, bias=bk_sb[D:P, m:m + 1], scale=1.0)

                def load_w(w_d):
                    view = w_d.ap().rearrange("(kc p) n -> kc p n", p=P)
                    tiles = []
                    for kc in range(KC):
                        wt = wpool.tile([P, CO], f32r, tag="w")
                        [nc.sync, nc.scalar, nc.gpsimd][kc % 3].dma_start(
                            wt[:], view[kc])
                        tiles.append(wt)
                    return tiles

                qk_stage(wq_t, bq_sb, write_qT)
                qk_stage(load_w(wk_d), bk_sb, write_kT)

                wv_t = load_w(wv_d)
                bvb_v = bvb_sb[:].rearrange("p (h d) -> p h d", d=D)
                for tt in range(TT):
                    ps = psB.tile([P, CO], dt32, tag="psB")
                    for kc in range(KC):
                        nc.tensor.matmul(
                            ps[:],
                            xts[kc][:, tt * P:(tt + 1) * P],
                            wv_t[kc][:],
                            start=(kc == 0), stop=(kc == KC - 1))
                    nc.vector.scalar_tensor_tensor(
                        out=vaug[:, tt, :, 0:D],
                        in0=ps[:].rearrange("p (h d) -> p h d", d=D),
                        scalar=1.0, in1=bvb_v,
                        op0=Alu.mult, op1=Alu.add)

            # ---------------- stage C: attention per head ----------------
            late_cm = tc.tile_pool(name="late", bufs=1)
            late = late_cm.__enter__()
            yT = late.tile([P, NP, T], f32r, tag="yT")
            with nc.named_scope("attn"), \
                 tc.tile_pool(name="ptp", bufs=2) as ptpool, \
                 tc.tile_pool(name="nrm", bufs=1) as nrmpool, \
                 tc.tile_pool(name="ysp", bufs=2) as yspool, \
                 tc.tile_pool(name="psS", bufs=2, space="PSUM") as psS, \
                 tc.tile_pool(name="psY", bufs=1, space="PSUM") as psY:
                for h in range(HC):
                    m, r0 = h // 2, (h % 2) * D
                    yt = psY.tile([MV, T], dt32, tag="yt")
                    for j in range(TT):
                        jb = j * P
                        span = T - jb
                        pt = ptpool.tile([P, span], bf16, tag="pt")
                        for sc in range(jb // WS, T // WS):
                            qs0 = max(WS * sc, jb)
                            sps = psS.tile([P, WS], dt32, tag="s")
                            for cg in range(qs0 // W, (WS * (sc + 1)) // W):
                                qs = max(W * cg, qs0)
                                w = W * (cg + 1) - qs
                                nc.tensor.matmul(
                                    sps[:, qs - WS * sc:qs - WS * sc + w],
                                    kT[:, h, jb:jb + P],
                                    qT[:, m, qs:qs + w],
                                    start=True, stop=True,
                                    skip_group_check=True)
                            nc.scalar.activation(
                                pt[:, qs0 - jb:WS * (sc + 1) - jb],
                                sps[:, qs0 - WS * sc:WS],
                                ActF.Exp, scale=scale)
                        nc.vector.tensor_mul(pt[:, 0:P], pt[:, 0:P], tri[:])
                        for cg in range(jb // W, NCH):
                            qs = max(W * cg, jb)
                            w = W * (cg + 1) - qs
                            last_j = (W * (cg + 1)) // P - 1
                            nc.tensor.matmul(
                                yt[:, qs:qs + w],
                                vaug[:, j, h, :],
                                pt[:, qs - jb:qs - jb + w],
                                start=(j == 0), stop=(j == last_j),
                                skip_group_check=True)
                    # copy PSUM accumulator out quickly to release it for the
                    # next head; the slow normalize chain then runs SBUF-side.
                    ys = yspool.tile([D + 1, T], dt32, tag="ys")
                    nc.vector.tensor_copy(ys[:], yt[0:D + 1, :])
                    nc.sync.dma_start(
                        lsc_d.ap().rearrange("(o t) -> o t", o=1),
                        ys[D:D + 1, :])
                    l128 = nrmpool.tile([P, T // P], dt32, tag="l128")
                    nc.gpsimd.dma_start(
                        l128[:], lsc_d.ap().rearrange("(p c) -> p c", p=P))
                    nc.vector.reciprocal(l128[:], l128[:])
                    nc.gpsimd.dma_start(
                        lsc2_d.ap().rearrange("(p c) -> p c", p=P), l128[:])
                    rcp = nrmpool.tile([1, T], dt32, tag="rcp")
                    nc.sync.dma_start(
                        rcp[:], lsc2_d.ap().rearrange("(o t) -> o t", o=1))
                    bc = nrmpool.tile([D, T], dt32, tag="bc")
                    nc.gpsimd.partition_broadcast(bc[:], rcp[:])
                    nc.vector.tensor_mul(yT[r0:r0 + D, m, :], ys[0:D, :], bc[:])

            # ---------------- stage E: output projection ----------------
            with nc.named_scope("proj"), \
                 tc.tile_pool(name="wpp", bufs=1) as wppool, \
                 tc.tile_pool(name="ost", bufs=3) as opool, \
                 tc.tile_pool(name="psO", bufs=4, space="PSUM") as psO:
                wpsb = wppool.tile([P, NP, C], f32r)
                nc.sync.dma_start(
                    wpsb[:], wp_d.ap().rearrange("(kt p) n -> p kt n", p=P))
                for tt in range(TT):
                    po = psO.tile([P, C], dt32, tag="o")
                    for kt in range(NP):
                        for nn in range(NW):
                            nc.tensor.matmul(
                                po[:, nn * WO:(nn + 1) * WO],
                                yT[:, kt, tt * P:(tt + 1) * P],
                                wpsb[:, kt, nn * WO:(nn + 1) * WO],
                                start=(kt == 0), stop=(kt == NP - 1),
                                skip_group_check=True)
                    ot = opool.tile([P, C], dt32, tag="ot")
                    nc.scalar.activation(ot[:], po[:], ActF.Copy)
                    [nc.sync, nc.scalar, nc.gpsimd][tt % 3].dma_start(
                        out_d.ap()[tt * P:(tt + 1) * P, :], ot[:])
            late_cm.__exit__(None, None, None)

    nc.compile()
    return nc


def make_core_inputs(x, W_attn, b_attn, W_proj, n_cores=8, HC=8, D=64):
    """Host-side sharding: per-core input dicts."""
    B, T, C = x.shape
    CO = HC * D
    NP = CO // P
    in_maps = []
    for c in range(n_cores):
        b = c // (n_cores // B)
        h0 = (c % (n_cores // B)) * HC
        lo = h0 * D
        bq = b_attn[lo:lo + CO]
        bk = b_attn[C + lo:C + lo + CO]
        bv = b_attn[2 * C + lo:2 * C + lo + CO]
        in_maps.append({
            "xt": np.ascontiguousarray(x[b].T),
            "wq": np.ascontiguousarray(W_attn[:, lo:lo + CO]),
            "wk": np.ascontiguousarray(W_attn[:, C + lo:C + lo + CO]),
            "wv": np.ascontiguousarray(W_attn[:, 2 * C + lo:2 * C + lo + CO]),
            "bq": np.ascontiguousarray(bq.reshape(NP, P).T),
            "bk": np.ascontiguousarray(bk.reshape(NP, P).T),
            "bvb": np.tile(bv[None, :], (P, 1)),
            "ones": np.ones((P, (T // P) * HC), _bf16_np()),
            "wp": np.ascontiguousarray(W_proj[lo:lo + CO, :]),
        })
    return in_maps


_CACHE = {}


def _get_program():
    if "nc" not in _CACHE:
        _CACHE["nc"] = build_program()
    return _CACHE["nc"]


def run_on_cores(x, W_attn, b_attn, W_proj, b_proj, trace=False):
    """Returns (full output [B,T,C], BassKernelResults)."""
    from concourse.bass_utils import run_bass_kernel_spmd

    x = np.asarray(x, np.float32)
    W_attn = np.asarray(W_attn, np.float32)
    b_attn = np.asarray(b_attn, np.float32)
    W_proj = np.asarray(W_proj, np.float32)
    b_proj = np.asarray(b_proj, np.float32)

    nc = _get_program()
    in_maps = make_core_inputs(x, W_attn, b_attn, W_proj)
    res = run_bass_kernel_spmd(nc, in_maps, core_ids=list(range(8)), trace=trace)
    B, T, C = x.shape
    out = np.empty((B, T, C), np.float32)
    for b in range(B):
        out[b] = (res.results[2 * b]["out"] + res.results[2 * b + 1]["out"]
                  + b_proj[None, :])
    return out, res


def kernel(x, W_attn, b_attn, W_proj, b_proj):
    out, _ = run_on_cores(x, W_attn, b_attn, W_proj, b_proj, trace=False)
    return out


# revision 35
# speedup vs baseline: 1.5303x; 1.0760x over previous
"""Causal self-attention (B=4, T=2048, C=1024, H=16) on 8 trn2 NeuronCores.

Sharding: core c -> batch b = c//2, heads h0 = (c%2)*8 .. h0+8 (tensor
parallel over heads: c_attn columns / c_proj rows split). Each core computes a
partial projection output [T, C]; the host sums the two partials per batch and
adds b_proj.

Device-side dataflow (all matmuls in float32r = full PE rate, fp32 data):
  - host passes x[b] pre-transposed as xt [C, T]
  - qT, kT  [C_head, T] computed with W_attn column-slices as stationary
  - v computed in natural [T, D] layout, augmented with a ones column so the
    PV matmul also produces the softmax denominator (row 64 of yT_aug)
  - S^T tiles [Tk=128, Tq<=512] = kT_tile^T . qT_chunk  (causal: only the
    lower triangle of S, i.e. Tq >= Tk tiles, is computed)
  - P~ = exp(S^T * 0.125) on ScalarE (no max-subtraction: scores are O(1));
    diagonal 128x128 blocks masked with an upper-triangular 0/1 mask
  - yT_aug [65, T] += v_aug_tile^T . P~  accumulated in PSUM over k-tiles
  - normalize: reciprocal of row 64, gpsimd partition-broadcast, DVE multiply
  - proj: out_tile [128, C] = yT_tile^T . W_proj_rows, streamed to DRAM
"""

import numpy as np

P = 128


def _bf16_np():
    import ml_dtypes
    return ml_dtypes.bfloat16


def build_program(T=2048, C=1024, HC=8, D=64, num_devices=8, trn="TRN2"):
    import concourse.mybir as mybir
    import concourse.tile as tile
    from concourse import bacc
    from concourse.masks import make_upper_triangular

    W = min(512, T)  # matmul moving-dim chunk
    WS = min(1024, T)  # score-PSUM superchunk (exp granularity)
    KC = C // P      # contraction tiles over C
    CO = HC * D      # this core's qkv channel block (512)
    NP = CO // P     # head pairs (2 heads of 64 = 1 partition tile)
    TT = T // P      # T tiles
    NCH = T // W     # T chunks
    WO = min(512, C)  # proj output column chunk
    NW = C // WO     # output column chunks
    dt32 = mybir.dt.float32
    f32r = mybir.dt.float32r
    bf16 = mybir.dt.bfloat16
    ActF = mybir.ActivationFunctionType
    Alu = mybir.AluOpType
    scale = 1.0 / float(np.sqrt(D))

    nc = bacc.Bacc(trn, target_bir_lowering=False, debug=False,
                   enable_asserts=False, num_devices=num_devices)

    xt_d = nc.dram_tensor("xt", [C, T], f32r, kind="ExternalInput")
    wq_d = nc.dram_tensor("wq", [C, CO], f32r, kind="ExternalInput")
    wk_d = nc.dram_tensor("wk", [C, CO], f32r, kind="ExternalInput")
    wv_d = nc.dram_tensor("wv", [C, CO], f32r, kind="ExternalInput")
    bq_d = nc.dram_tensor("bq", [P, NP], dt32, kind="ExternalInput")
    bk_d = nc.dram_tensor("bk", [P, NP], dt32, kind="ExternalInput")
    bvb_d = nc.dram_tensor("bvb", [P, CO], dt32, kind="ExternalInput")
    ones_d = nc.dram_tensor("ones", [P, TT * HC], bf16, kind="ExternalInput")
    wp_d = nc.dram_tensor("wp", [CO, C], f32r, kind="ExternalInput")
    out_d = nc.dram_tensor("out", [T, C], dt32, kind="ExternalOutput")
    lsc_d = nc.dram_tensor("lsc", [T], dt32)
    lsc2_d = nc.dram_tensor("lsc2", [T], dt32)

    with tile.TileContext(nc) as tc:
        with tc.tile_pool(name="const", bufs=1) as cpool, \
             tc.tile_pool(name="pers", bufs=1) as pers:
            tri = cpool.tile([P, P], bf16)
            make_upper_triangular(nc, tri[:], val=1.0, diag=True)
            bq_sb = cpool.tile([P, NP], dt32)
            nc.sync.dma_start(bq_sb[:], bq_d.ap())
            bk_sb = cpool.tile([P, NP], dt32)
            nc.sync.dma_start(bk_sb[:], bk_d.ap())
            bvb_sb = cpool.tile([P, CO], dt32)
            nc.sync.dma_start(bvb_sb[:], bvb_d.ap())

            MV = 96  # PV stationary columns (3 full 32-col PE groups)
            qT = pers.tile([P, NP, T], bf16, tag="qT")
            kT = pers.tile([P, HC, T], bf16, tag="kT")
            vaug = pers.tile([P, TT, HC, MV], bf16, tag="vaug")
            nc.vector.memset(kT[:], 0.0)
            nc.vector.memset(vaug[:], 0.0)
            nc.sync.dma_start(
                vaug[:, :, :, D],
                ones_d.ap().rearrange("p (a b) -> p a b", b=HC))

            # ---------------- stage B: qkv projections ----------------
            with nc.named_scope("qkv"), \
                 tc.tile_pool(name="xtp", bufs=KC) as xpool, \
                 tc.tile_pool(name="wp_in", bufs=KC) as wpool, \
                 tc.tile_pool(name="psB", bufs=2, space="PSUM") as psB:
                xt_view = xt_d.ap().rearrange("(kc p) t -> kc p t", p=P)
                wq_view = wq_d.ap().rearrange("(kc p) n -> kc p n", p=P)
                xts = []
                wq_t = []
                dmae = [nc.sync, nc.scalar, nc.gpsimd]
                for kc in range(KC):
                    xte = xpool.tile([P, T], f32r, tag="xt")
                    dmae[kc % 3].dma_start(xte[:], xt_view[kc])
                    xts.append(xte)
                    wt = wpool.tile([P, CO], f32r, tag="w")
                    dmae[(kc + 1) % 3].dma_start(wt[:], wq_view[kc])
                    wq_t.append(wt)

                def qk_stage(w_tiles, bias_sb, write_out):
                    for half in range((NP + 1) // 2):
                        ms = [m for m in (2 * half, 2 * half + 1) if m < NP]
                        pss = {}
                        for m in ms:
                            ps_m = psB.tile([P, T], dt32, tag="psB")
                            pss[m] = ps_m
                        for kc in range(KC):
                            for m in ms:
                                for cg in range(NCH):
                                    nc.tensor.matmul(
                                        pss[m][:, cg * W:(cg + 1) * W],
                                        w_tiles[kc][:, m * P:(m + 1) * P],
                                        xts[kc][:, cg * W:(cg + 1) * W],
                                        start=(kc == 0), stop=(kc == KC - 1),
                                        skip_group_check=True)
                        for m in ms:
                            write_out(m, pss[m])

                def write_qT(m, ps):
                    nc.scalar.activation(
                        qT[:, m, :], ps[:],
                        ActF.Identity, bias=bq_sb[:, m:m + 1], scale=1.0)

                def write_kT(m, ps):
                    # unpacked per-head, sibling rows stay zero
                    nc.scalar.activation(
                        kT[0:D, 2 * m, :], ps[0:D, :],
                        ActF.Identity, bias=bk_sb[0:D, m:m + 1], scale=1.0)
                    nc.scalar.activation(
                        kT[D:P, 2 * m + 1, :], ps[D:P, :],
                        ActF.Identity, bias=bk_sb[D:P, m:m + 1], scale=1.0)

                def load_w(w_d):
                    view = w_d.ap().rearrange("(kc p) n -> kc p n", p=P)
                    tiles = []
                    for kc in range(KC):
                        wt = wpool.tile([P, CO], f32r, tag="w")
                        [nc.sync, nc.scalar, nc.gpsimd][kc % 3].dma_start(
                            wt[:], view[kc])
                        tiles.append(wt)
                    return tiles

                qk_stage(wq_t, bq_sb, write_qT)
                qk_stage(load_w(wk_d), bk_sb, write_kT)

                wv_t = load_w(wv_d)
                bvb_v = bvb_sb[:].rearrange("p (h d) -> p h d", d=D)
                for tt in range(TT):
                    ps = psB.tile([P, CO], dt32, tag="psB")
                    for kc in range(KC):
                        nc.tensor.matmul(
                            ps[:],
                            xts[kc][:, tt * P:(tt + 1) * P],
                            wv_t[kc][:],
                            start=(kc == 0), stop=(kc == KC - 1))
                    nc.vector.scalar_tensor_tensor(
                        out=vaug[:, tt, :, 0:D],
                        in0=ps[:].rearrange("p (h d) -> p h d", d=D),
                        scalar=1.0, in1=bvb_v,
                        op0=Alu.mult, op1=Alu.add)

            # ---------------- stage C: attention per head ----------------
            late_cm = tc.tile_pool(name="late", bufs=1)
            late = late_cm.__enter__()
            yT = late.tile([P, NP, T], f32r, tag="yT")
            with nc.named_scope("attn"), \
                 tc.tile_pool(name="ptp", bufs=2) as ptpool, \
                 tc.tile_pool(name="nrm", bufs=1) as nrmpool, \
                 tc.tile_pool(name="ysp", bufs=2) as yspool, \
                 tc.tile_pool(name="psS", bufs=2, space="PSUM") as psS, \
                 tc.tile_pool(name="psY", bufs=1, space="PSUM") as psY:
                yts = {}

                def emit_s(h, j):
                    m = h // 2
                    jb = j * P
                    span = T - jb
                    pt = ptpool.tile([P, span], bf16, tag="pt")
                    for sc in range(jb // WS, T // WS):
                        qs0 = max(WS * sc, jb)
                        sps = psS.tile([P, WS], dt32, tag="s")
                        for cg in range(qs0 // W, (WS * (sc + 1)) // W):
                            qs = max(W * cg, qs0)
                            w = W * (cg + 1) - qs
                            nc.tensor.matmul(
                                sps[:, qs - WS * sc:qs - WS * sc + w],
                                kT[:, h, jb:jb + P],
                                qT[:, m, qs:qs + w],
                                start=True, stop=True,
                                skip_group_check=True)
                        nc.scalar.activation(
                            pt[:, qs0 - jb:WS * (sc + 1) - jb],
                            sps[:, qs0 - WS * sc:WS],
                            ActF.Exp, scale=scale)
                    nc.vector.tensor_mul(pt[:, 0:P], pt[:, 0:P], tri[:])
                    return pt

                def emit_pv(h, j, pt):
                    jb = j * P
                    if j == 0:
                        yt_new = psY.tile([MV, T], dt32, tag="yt")
                        yts[h] = yt_new
                    yt = yts[h]
                    for cg in range(jb // W, NCH):
                        qs = max(W * cg, jb)
                        w = W * (cg + 1) - qs
                        last_j = (W * (cg + 1)) // P - 1
                        nc.tensor.matmul(
                            yt[:, qs:qs + w],
                            vaug[:, j, h, :],
                            pt[:, qs - jb:qs - jb + w],
                            start=(j == 0), stop=(j == last_j),
                            skip_group_check=True)

                def finish_head(h):
                    # copy PSUM accumulator out quickly to release it for the
                    # next head; the slow normalize chain then runs SBUF-side.
                    m, r0 = h // 2, (h % 2) * D
                    yt = yts.pop(h)
                    ys = yspool.tile([D + 1, T], dt32, tag="ys")
                    nc.vector.tensor_copy(ys[:], yt[0:D + 1, :])
                    nc.sync.dma_start(
                        lsc_d.ap().rearrange("(o t) -> o t", o=1),
                        ys[D:D + 1, :])
                    l128 = nrmpool.tile([P, T // P], dt32, tag="l128")
                    nc.gpsimd.dma_start(
                        l128[:], lsc_d.ap().rearrange("(p c) -> p c", p=P))
                    nc.vector.reciprocal(l128[:], l128[:])
                    nc.gpsimd.dma_start(
                        lsc2_d.ap().rearrange("(p c) -> p c", p=P), l128[:])
                    rcp = nrmpool.tile([1, T], dt32, tag="rcp")
                    nc.sync.dma_start(
                        rcp[:], lsc2_d.ap().rearrange("(o t) -> o t", o=1))
                    bc = nrmpool.tile([D, T], dt32, tag="bc")
                    nc.gpsimd.partition_broadcast(bc[:], rcp[:])
                    nc.vector.tensor_mul(yT[r0:r0 + D, m, :], ys[0:D, :], bc[:])

                pending = None
                for h in range(HC):
                    for j in range(TT):
                        pt = emit_s(h, j)
                        if pending is not None:
                            ph, pj, ppt = pending
                            emit_pv(ph, pj, ppt)
                            if pj == TT - 1:
                                finish_head(ph)
                        pending = (h, j, pt)
                ph, pj, ppt = pending
                emit_pv(ph, pj, ppt)
                finish_head(ph)

            # ---------------- stage E: output projection ----------------
            with nc.named_scope("proj"), \
                 tc.tile_pool(name="wpp", bufs=1) as wppool, \
                 tc.tile_pool(name="ost", bufs=3) as opool, \
                 tc.tile_pool(name="psO", bufs=4, space="PSUM") as psO:
                wpsb = wppool.tile([P, NP, C], f32r)
                nc.sync.dma_start(
                    wpsb[:], wp_d.ap().rearrange("(kt p) n -> p kt n", p=P))
                for tt in range(TT):
                    po = psO.tile([P, C], dt32, tag="o")
                    for kt in range(NP):
                        for nn in range(NW):
                            nc.tensor.matmul(
                                po[:, nn * WO:(nn + 1) * WO],
                                yT[:, kt, tt * P:(tt + 1) * P],
                                wpsb[:, kt, nn * WO:(nn + 1) * WO],
                                start=(kt == 0), stop=(kt == NP - 1),
                                skip_group_check=True)
                    ot = opool.tile([P, C], dt32, tag="ot")
                    nc.scalar.activation(ot[:], po[:], ActF.Copy)
                    [nc.sync, nc.scalar, nc.gpsimd][tt % 3].dma_start(
                        out_d.ap()[tt * P:(tt + 1) * P, :], ot[:])
            late_cm.__exit__(None, None, None)

    nc.compile()
    return nc


def make_core_inputs(x, W_attn, b_attn, W_proj, n_cores=8, HC=8, D=64):
    """Host-side sharding: per-core input dicts."""
    B, T, C = x.shape
    CO = HC * D
    NP = CO // P
    in_maps = []
    for c in range(n_cores):
        b = c // (n_cores // B)
        h0 = (c % (n_cores // B)) * HC
        lo = h0 * D
        bq = b_attn[lo:lo + CO]
        bk = b_attn[C + lo:C + lo + CO]
        bv = b_attn[2 * C + lo:2 * C + lo + CO]
        in_maps.append({
            "xt": np.ascontiguousarray(x[b].T),
            "wq": np.ascontiguousarray(W_attn[:, lo:lo + CO]),
            "wk": np.ascontiguousarray(W_attn[:, C + lo:C + lo + CO]),
            "wv": np.ascontiguousarray(W_attn[:, 2 * C + lo:2 * C + lo + CO]),
            "bq": np.ascontiguousarray(bq.reshape(NP, P).T),
            "bk": np.ascontiguousarray(bk.reshape(NP, P).T),
            "bvb": np.tile(bv[None, :], (P, 1)),
            "ones": np.ones((P, (T // P) * HC), _bf16_np()),
            "wp": np.ascontiguousarray(W_proj[lo:lo + CO, :]),
        })
    return in_maps


_CACHE = {}


def _get_program():
    if "nc" not in _CACHE:
        _CACHE["nc"] = build_program()
    return _CACHE["nc"]


def run_on_cores(x, W_attn, b_attn, W_proj, b_proj, trace=False):
    """Returns (full output [B,T,C], BassKernelResults)."""
    from concourse.bass_utils import run_bass_kernel_spmd

    x = np.asarray(x, np.float32)
    W_attn = np.asarray(W_attn, np.float32)
    b_attn = np.asarray(b_attn, np.float32)
    W_proj = np.asarray(W_proj, np.float32)
    b_proj = np.asarray(b_proj, np.float32)

    nc = _get_program()
    in_maps = make_core_inputs(x, W_attn, b_attn, W_proj)
    res = run_bass_kernel_spmd(nc, in_maps, core_ids=list(range(8)), trace=trace)
    B, T, C = x.shape
    out = np.empty((B, T, C), np.float32)
    for b in range(B):
        out[b] = (res.results[2 * b]["out"] + res.results[2 * b + 1]["out"]
                  + b_proj[None, :])
    return out, res


def kernel(x, W_attn, b_attn, W_proj, b_proj):
    out, _ = run_on_cores(x, W_attn, b_attn, W_proj, b_proj, trace=False)
    return out


# revision 36
# speedup vs baseline: 1.6650x; 1.0880x over previous
"""Causal self-attention (B=4, T=2048, C=1024, H=16) on 8 trn2 NeuronCores.

Sharding: core c -> batch b = c//2, heads h0 = (c%2)*8 .. h0+8 (tensor
parallel over heads: c_attn columns / c_proj rows split). Each core computes a
partial projection output [T, C]; the host sums the two partials per batch and
adds b_proj.

Device-side dataflow (all matmuls in float32r = full PE rate, fp32 data):
  - host passes x[b] pre-transposed as xt [C, T]
  - qT, kT  [C_head, T] computed with W_attn column-slices as stationary
  - v computed in natural [T, D] layout, augmented with a ones column so the
    PV matmul also produces the softmax denominator (row 64 of yT_aug)
  - S^T tiles [Tk=128, Tq<=512] = kT_tile^T . qT_chunk  (causal: only the
    lower triangle of S, i.e. Tq >= Tk tiles, is computed)
  - P~ = exp(S^T * 0.125) on ScalarE (no max-subtraction: scores are O(1));
    diagonal 128x128 blocks masked with an upper-triangular 0/1 mask
  - yT_aug [65, T] += v_aug_tile^T . P~  accumulated in PSUM over k-tiles
  - normalize: reciprocal of row 64, gpsimd partition-broadcast, DVE multiply
  - proj: out_tile [128, C] = yT_tile^T . W_proj_rows, streamed to DRAM
"""

import numpy as np

P = 128


def _bf16_np():
    import ml_dtypes
    return ml_dtypes.bfloat16


def build_program(T=2048, C=1024, HC=8, D=64, num_devices=8, trn="TRN2"):
    import concourse.mybir as mybir
    import concourse.tile as tile
    from concourse import bacc
    from concourse.masks import make_upper_triangular

    W = min(512, T)  # matmul moving-dim chunk
    WS = min(1024, T)  # score-PSUM superchunk (exp granularity)
    KC = C // P      # contraction tiles over C
    CO = HC * D      # this core's qkv channel block (512)
    NP = CO // P     # head pairs (2 heads of 64 = 1 partition tile)
    TT = T // P      # T tiles
    NCH = T // W     # T chunks
    WO = min(512, C)  # proj output column chunk
    NW = C // WO     # output column chunks
    dt32 = mybir.dt.float32
    f32r = mybir.dt.float32r
    bf16 = mybir.dt.bfloat16
    ActF = mybir.ActivationFunctionType
    Alu = mybir.AluOpType
    scale = 1.0 / float(np.sqrt(D))

    nc = bacc.Bacc(trn, target_bir_lowering=False, debug=False,
                   enable_asserts=False, num_devices=num_devices)

    xt_d = nc.dram_tensor("xt", [C, T], f32r, kind="ExternalInput")
    wq_d = nc.dram_tensor("wq", [C, CO], f32r, kind="ExternalInput")
    wk_d = nc.dram_tensor("wk", [C, CO], f32r, kind="ExternalInput")
    wv_d = nc.dram_tensor("wv", [C, CO], f32r, kind="ExternalInput")
    bq_d = nc.dram_tensor("bq", [P, NP], dt32, kind="ExternalInput")
    bk_d = nc.dram_tensor("bk", [P, NP], dt32, kind="ExternalInput")
    bvb_d = nc.dram_tensor("bvb", [P, CO], dt32, kind="ExternalInput")
    ones_d = nc.dram_tensor("ones", [P, TT * HC], bf16, kind="ExternalInput")
    wp_d = nc.dram_tensor("wp", [CO, C], f32r, kind="ExternalInput")
    out_d = nc.dram_tensor("out", [T, C], dt32, kind="ExternalOutput")
    lsc_d = nc.dram_tensor("lsc", [T], dt32)
    lsc2_d = nc.dram_tensor("lsc2", [T], dt32)

    with tile.TileContext(nc) as tc:
        with tc.tile_pool(name="const", bufs=1) as cpool, \
             tc.tile_pool(name="pers", bufs=1) as pers:
            tri = cpool.tile([P, P], bf16)
            make_upper_triangular(nc, tri[:], val=1.0, diag=True)
            bq_sb = cpool.tile([P, NP], dt32)
            nc.sync.dma_start(bq_sb[:], bq_d.ap())
            bk_sb = cpool.tile([P, NP], dt32)
            nc.sync.dma_start(bk_sb[:], bk_d.ap())
            bvb_sb = cpool.tile([P, CO], dt32)
            nc.sync.dma_start(bvb_sb[:], bvb_d.ap())

            MV = 96  # PV stationary columns (3 full 32-col PE groups)
            qT = pers.tile([P, NP, T], bf16, tag="qT")
            kT = pers.tile([P, HC, T], bf16, tag="kT")
            vaug = pers.tile([P, TT, HC, MV], bf16, tag="vaug")
            nc.vector.memset(kT[:], 0.0)
            nc.vector.memset(vaug[:], 0.0)
            nc.sync.dma_start(
                vaug[:, :, :, D],
                ones_d.ap().rearrange("p (a b) -> p a b", b=HC))

            # ---------------- stage B: qkv projections ----------------
            with nc.named_scope("qkv"), \
                 tc.tile_pool(name="xtp", bufs=KC * NCH) as xpool, \
                 tc.tile_pool(name="wp_in", bufs=KC) as wpool, \
                 tc.tile_pool(name="psB", bufs=2, space="PSUM") as psB:
                xt_view = xt_d.ap().rearrange("(kc p) t -> kc p t", p=P)
                wq_view = wq_d.ap().rearrange("(kc p) n -> kc p n", p=P)
                xts = []
                wq_t = []
                dmae = [nc.sync, nc.scalar, nc.gpsimd]
                di = 0
                for kc in range(KC):
                    wt = wpool.tile([P, CO], f32r, tag="w")
                    dmae[di % 3].dma_start(wt[:], wq_view[kc])
                    di += 1
                    wq_t.append(wt)
                    row = []
                    for cg in range(NCH):
                        xtc = xpool.tile([P, W], f32r, tag="xt")
                        dmae[di % 3].dma_start(
                            xtc[:], xt_view[kc][:, cg * W:(cg + 1) * W])
                        di += 1
                        row.append(xtc)
                    xts.append(row)

                def qk_stage(w_tiles, bias_sb, write_out):
                    for half in range((NP + 1) // 2):
                        ms = [m for m in (2 * half, 2 * half + 1) if m < NP]
                        pss = {}
                        for m in ms:
                            ps_m = psB.tile([P, T], dt32, tag="psB")
                            pss[m] = ps_m
                        for kc in range(KC):
                            for m in ms:
                                for cg in range(NCH):
                                    nc.tensor.matmul(
                                        pss[m][:, cg * W:(cg + 1) * W],
                                        w_tiles[kc][:, m * P:(m + 1) * P],
                                        xts[kc][cg][:],
                                        start=(kc == 0), stop=(kc == KC - 1),
                                        skip_group_check=True)
                        for m in ms:
                            write_out(m, pss[m])

                def write_qT(m, ps):
                    nc.scalar.activation(
                        qT[:, m, :], ps[:],
                        ActF.Identity, bias=bq_sb[:, m:m + 1], scale=1.0)

                def write_kT(m, ps):
                    # unpacked per-head, sibling rows stay zero
                    nc.scalar.activation(
                        kT[0:D, 2 * m, :], ps[0:D, :],
                        ActF.Identity, bias=bk_sb[0:D, m:m + 1], scale=1.0)
                    nc.scalar.activation(
                        kT[D:P, 2 * m + 1, :], ps[D:P, :],
                        ActF.Identity, bias=bk_sb[D:P, m:m + 1], scale=1.0)

                def load_w(w_d):
                    view = w_d.ap().rearrange("(kc p) n -> kc p n", p=P)
                    tiles = []
                    for kc in range(KC):
                        wt = wpool.tile([P, CO], f32r, tag="w")
                        [nc.sync, nc.scalar, nc.gpsimd][kc % 3].dma_start(
                            wt[:], view[kc])
                        tiles.append(wt)
                    return tiles

                qk_stage(wq_t, bq_sb, write_qT)
                qk_stage(load_w(wk_d), bk_sb, write_kT)

                wv_t = load_w(wv_d)
                bvb_v = bvb_sb[:].rearrange("p (h d) -> p h d", d=D)
                for tt in range(TT):
                    ps = psB.tile([P, CO], dt32, tag="psB")
                    for kc in range(KC):
                        nc.tensor.matmul(
                            ps[:],
                            xts[kc][tt * P // W][:, (tt * P % W):(tt * P % W) + P],
                            wv_t[kc][:],
                            start=(kc == 0), stop=(kc == KC - 1))
                    nc.vector.scalar_tensor_tensor(
                        out=vaug[:, tt, :, 0:D],
                        in0=ps[:].rearrange("p (h d) -> p h d", d=D),
                        scalar=1.0, in1=bvb_v,
                        op0=Alu.mult, op1=Alu.add)

            # ---------------- stage C: attention per head ----------------
            late_cm = tc.tile_pool(name="late", bufs=1)
            late = late_cm.__enter__()
            yT = late.tile([P, NP, T], f32r, tag="yT")
            with nc.named_scope("attn"), \
                 tc.tile_pool(name="ptp", bufs=3) as ptpool, \
                 tc.tile_pool(name="nrm", bufs=1) as nrmpool, \
                 tc.tile_pool(name="ysp", bufs=2) as yspool, \
                 tc.tile_pool(name="psS", bufs=2, space="PSUM") as psS, \
                 tc.tile_pool(name="psY", bufs=1, space="PSUM") as psY:
                yts = {}

                def emit_s(h, j):
                    m = h // 2
                    jb = j * P
                    span = T - jb
                    pt = ptpool.tile([P, span], bf16, tag="pt")
                    for sc in range(jb // WS, T // WS):
                        qs0 = max(WS * sc, jb)
                        sps = psS.tile([P, WS], dt32, tag="s")
                        for cg in range(qs0 // W, (WS * (sc + 1)) // W):
                            qs = max(W * cg, qs0)
                            w = W * (cg + 1) - qs
                            nc.tensor.matmul(
                                sps[:, qs - WS * sc:qs - WS * sc + w],
                                kT[:, h, jb:jb + P],
                                qT[:, m, qs:qs + w],
                                start=True, stop=True,
                                skip_group_check=True)
                        nc.scalar.activation(
                            pt[:, qs0 - jb:WS * (sc + 1) - jb],
                            sps[:, qs0 - WS * sc:WS],
                            ActF.Exp, scale=scale)
                    nc.vector.tensor_mul(pt[:, 0:P], pt[:, 0:P], tri[:])
                    return pt

                def emit_pv(h, j, pt):
                    jb = j * P
                    if j == 0:
                        yt_new = psY.tile([MV, T], dt32, tag="yt")
                        yts[h] = yt_new
                    yt = yts[h]
                    for cg in range(jb // W, NCH):
                        qs = max(W * cg, jb)
                        w = W * (cg + 1) - qs
                        last_j = (W * (cg + 1)) // P - 1
                        nc.tensor.matmul(
                            yt[:, qs:qs + w],
                            vaug[:, j, h, :],
                            pt[:, qs - jb:qs - jb + w],
                            start=(j == 0), stop=(j == last_j),
                            skip_group_check=True)

                def finish_head(h):
                    # copy PSUM accumulator out quickly to release it for the
                    # next head; the slow normalize chain then runs SBUF-side.
                    m, r0 = h // 2, (h % 2) * D
                    yt = yts.pop(h)
                    ys = yspool.tile([D + 1, T], dt32, tag="ys")
                    nc.vector.tensor_copy(ys[:], yt[0:D + 1, :])
                    nc.sync.dma_start(
                        lsc_d.ap().rearrange("(o t) -> o t", o=1),
                        ys[D:D + 1, :])
                    l128 = nrmpool.tile([P, T // P], dt32, tag="l128")
                    nc.gpsimd.dma_start(
                        l128[:], lsc_d.ap().rearrange("(p c) -> p c", p=P))
                    nc.vector.reciprocal(l128[:], l128[:])
                    nc.gpsimd.dma_start(
                        lsc2_d.ap().rearrange("(p c) -> p c", p=P), l128[:])
                    bc = nrmpool.tile([D, T], dt32, tag="bc")
                    nc.sync.dma_start(
                        bc[:],
                        lsc2_d.ap().rearrange("(o t) -> o t", o=1).broadcast_to(
                            [D, T]))
                    nc.vector.tensor_mul(yT[r0:r0 + D, m, :], ys[0:D, :], bc[:])

                pending = []
                for h in range(HC):
                    for j in range(TT):
                        pt = emit_s(h, j)
                        pending.append((h, j, pt))
                        if len(pending) > 2:
                            ph, pj, ppt = pending.pop(0)
                            emit_pv(ph, pj, ppt)
                            if pj == TT - 1:
                                finish_head(ph)
                for ph, pj, ppt in pending:
                    emit_pv(ph, pj, ppt)
                    if pj == TT - 1:
                        finish_head(ph)

            # ---------------- stage E: output projection ----------------
            with nc.named_scope("proj"), \
                 tc.tile_pool(name="wpp", bufs=1) as wppool, \
                 tc.tile_pool(name="ost", bufs=3) as opool, \
                 tc.tile_pool(name="psO", bufs=4, space="PSUM") as psO:
                wpsb = wppool.tile([P, NP, C], f32r)
                nc.sync.dma_start(
                    wpsb[:], wp_d.ap().rearrange("(kt p) n -> p kt n", p=P))
                for tt in range(TT):
                    po = psO.tile([P, C], dt32, tag="o")
                    for kt in range(NP):
                        for nn in range(NW):
                            nc.tensor.matmul(
                                po[:, nn * WO:(nn + 1) * WO],
                                yT[:, kt, tt * P:(tt + 1) * P],
                                wpsb[:, kt, nn * WO:(nn + 1) * WO],
                                start=(kt == 0), stop=(kt == NP - 1),
                                skip_group_check=True)
                    ot = opool.tile([P, C], dt32, tag="ot")
                    nc.scalar.activation(ot[:], po[:], ActF.Copy)
                    [nc.sync, nc.scalar, nc.gpsimd][tt % 3].dma_start(
                        out_d.ap()[tt * P:(tt + 1) * P, :], ot[:])
            late_cm.__exit__(None, None, None)

    nc.compile()
    return nc


def make_core_inputs(x, W_attn, b_attn, W_proj, n_cores=8, HC=8, D=64):
    """Host-side sharding: per-core input dicts."""
    B, T, C = x.shape
    CO = HC * D
    NP = CO // P
    in_maps = []
    for c in range(n_cores):
        b = c // (n_cores // B)
        h0 = (c % (n_cores // B)) * HC
        lo = h0 * D
        bq = b_attn[lo:lo + CO]
        bk = b_attn[C + lo:C + lo + CO]
        bv = b_attn[2 * C + lo:2 * C + lo + CO]
        in_maps.append({
            "xt": np.ascontiguousarray(x[b].T),
            "wq": np.ascontiguousarray(W_attn[:, lo:lo + CO]),
            "wk": np.ascontiguousarray(W_attn[:, C + lo:C + lo + CO]),
            "wv": np.ascontiguousarray(W_attn[:, 2 * C + lo:2 * C + lo + CO]),
            "bq": np.ascontiguousarray(bq.reshape(NP, P).T),
            "bk": np.ascontiguousarray(bk.reshape(NP, P).T),
            "bvb": np.tile(bv[None, :], (P, 1)),
            "ones": np.ones((P, (T // P) * HC), _bf16_np()),
            "wp": np.ascontiguousarray(W_proj[lo:lo + CO, :]),
        })
    return in_maps


_CACHE = {}


def _get_program():
    if "nc" not in _CACHE:
        _CACHE["nc"] = build_program()
    return _CACHE["nc"]


def run_on_cores(x, W_attn, b_attn, W_proj, b_proj, trace=False):
    """Returns (full output [B,T,C], BassKernelResults)."""
    from concourse.bass_utils import run_bass_kernel_spmd

    x = np.asarray(x, np.float32)
    W_attn = np.asarray(W_attn, np.float32)
    b_attn = np.asarray(b_attn, np.float32)
    W_proj = np.asarray(W_proj, np.float32)
    b_proj = np.asarray(b_proj, np.float32)

    nc = _get_program()
    in_maps = make_core_inputs(x, W_attn, b_attn, W_proj)
    res = run_bass_kernel_spmd(nc, in_maps, core_ids=list(range(8)), trace=trace)
    B, T, C = x.shape
    out = np.empty((B, T, C), np.float32)
    for b in range(B):
        out[b] = (res.results[2 * b]["out"] + res.results[2 * b + 1]["out"]
                  + b_proj[None, :])
    return out, res


def kernel(x, W_attn, b_attn, W_proj, b_proj):
    out, _ = run_on_cores(x, W_attn, b_attn, W_proj, b_proj, trace=False)
    return out


# revision 39
# speedup vs baseline: 1.7341x; 1.0415x over previous
"""Causal self-attention (B=4, T=2048, C=1024, H=16) on 8 trn2 NeuronCores.

Sharding: core c -> batch b = c//2, heads h0 = (c%2)*8 .. h0+8 (tensor
parallel over heads: c_attn columns / c_proj rows split). Each core computes a
partial projection output [T, C]; the host sums the two partials per batch and
adds b_proj.

Device-side dataflow (all matmuls in float32r = full PE rate, fp32 data):
  - host passes x[b] pre-transposed as xt [C, T]
  - qT, kT  [C_head, T] computed with W_attn column-slices as stationary
  - v computed in natural [T, D] layout, augmented with a ones column so the
    PV matmul also produces the softmax denominator (row 64 of yT_aug)
  - S^T tiles [Tk=128, Tq<=512] = kT_tile^T . qT_chunk  (causal: only the
    lower triangle of S, i.e. Tq >= Tk tiles, is computed)
  - P~ = exp(S^T * 0.125) on ScalarE (no max-subtraction: scores are O(1));
    diagonal 128x128 blocks masked with an upper-triangular 0/1 mask
  - yT_aug [65, T] += v_aug_tile^T . P~  accumulated in PSUM over k-tiles
  - normalize: reciprocal of row 64, gpsimd partition-broadcast, DVE multiply
  - proj: out_tile [128, C] = yT_tile^T . W_proj_rows, streamed to DRAM
"""

import numpy as np

P = 128


def _bf16_np():
    import ml_dtypes
    return ml_dtypes.bfloat16


def build_program(T=2048, C=1024, HC=8, D=64, num_devices=8, trn="TRN2"):
    import concourse.mybir as mybir
    import concourse.tile as tile
    from concourse import bacc
    from concourse.masks import make_upper_triangular

    W = min(512, T)  # matmul moving-dim chunk
    WS = min(1024, T)  # score-PSUM superchunk (exp granularity)
    KC = C // P      # contraction tiles over C
    CO = HC * D      # this core's qkv channel block (512)
    NP = CO // P     # head pairs (2 heads of 64 = 1 partition tile)
    TT = T // P      # T tiles
    NCH = T // W     # T chunks
    WO = min(512, C)  # proj output column chunk
    NW = C // WO     # output column chunks
    dt32 = mybir.dt.float32
    f32r = mybir.dt.float32r
    bf16 = mybir.dt.bfloat16
    ActF = mybir.ActivationFunctionType
    Alu = mybir.AluOpType
    scale = 1.0 / float(np.sqrt(D))

    nc = bacc.Bacc(trn, target_bir_lowering=False, debug=False,
                   enable_asserts=False, num_devices=num_devices)

    xt_d = nc.dram_tensor("xt", [C, T], bf16, kind="ExternalInput")
    wq_d = nc.dram_tensor("wq", [C, CO], bf16, kind="ExternalInput")
    wk_d = nc.dram_tensor("wk", [C, CO], bf16, kind="ExternalInput")
    wv_d = nc.dram_tensor("wv", [C, CO], bf16, kind="ExternalInput")
    bq_d = nc.dram_tensor("bq", [P, NP], dt32, kind="ExternalInput")
    bk_d = nc.dram_tensor("bk", [P, NP], dt32, kind="ExternalInput")
    bvb_d = nc.dram_tensor("bvb", [P, CO], dt32, kind="ExternalInput")
    ones_d = nc.dram_tensor("ones", [P, TT * HC], bf16, kind="ExternalInput")
    wp_d = nc.dram_tensor("wp", [CO, C], f32r, kind="ExternalInput")
    out_d = nc.dram_tensor("out", [T, C], dt32, kind="ExternalOutput")
    lsc_d = nc.dram_tensor("lsc", [T], dt32)
    lsc2_d = nc.dram_tensor("lsc2", [T], dt32)

    with tile.TileContext(nc) as tc:
        with tc.tile_pool(name="const", bufs=1) as cpool, \
             tc.tile_pool(name="pers", bufs=1) as pers:
            tri = cpool.tile([P, P], bf16)
            make_upper_triangular(nc, tri[:], val=1.0, diag=True)
            bq_sb = cpool.tile([P, NP], dt32)
            nc.sync.dma_start(bq_sb[:], bq_d.ap())
            bk_sb = cpool.tile([P, NP], dt32)
            nc.sync.dma_start(bk_sb[:], bk_d.ap())
            bvb_sb = cpool.tile([P, CO], dt32)
            nc.sync.dma_start(bvb_sb[:], bvb_d.ap())

            MV = 96  # PV stationary columns (3 full 32-col PE groups)
            qT = pers.tile([P, NP, T], bf16, tag="qT")
            kT = pers.tile([P, HC, T], bf16, tag="kT")
            vaug = pers.tile([P, TT, HC, MV], bf16, tag="vaug")
            nc.vector.memset(kT[:], 0.0)
            nc.vector.memset(vaug[:], 0.0)
            nc.sync.dma_start(
                vaug[:, :, :, D],
                ones_d.ap().rearrange("p (a b) -> p a b", b=HC))

            # ---------------- stage B: qkv projections ----------------
            with nc.named_scope("qkv"), \
                 tc.tile_pool(name="xtp", bufs=KC * NCH) as xpool, \
                 tc.tile_pool(name="wp_in", bufs=KC) as wpool, \
                 tc.tile_pool(name="psB", bufs=2, space="PSUM") as psB:
                xt_view = xt_d.ap().rearrange("(kc p) t -> kc p t", p=P)
                wq_view = wq_d.ap().rearrange("(kc p) n -> kc p n", p=P)
                xts = []
                wq_t = []
                dmae = [nc.sync, nc.scalar, nc.gpsimd]
                di = 0
                for kc in range(KC):
                    wt = wpool.tile([P, CO], bf16, tag="w")
                    dmae[di % 3].dma_start(wt[:], wq_view[kc])
                    di += 1
                    wq_t.append(wt)
                    row = []
                    for cg in range(NCH):
                        xtc = xpool.tile([P, W], bf16, tag="xt")
                        dmae[di % 3].dma_start(
                            xtc[:], xt_view[kc][:, cg * W:(cg + 1) * W])
                        di += 1
                        row.append(xtc)
                    xts.append(row)

                def qk_stage(w_tiles, bias_sb, write_out):
                    for half in range((NP + 1) // 2):
                        ms = [m for m in (2 * half, 2 * half + 1) if m < NP]
                        pss = {}
                        for m in ms:
                            ps_m = psB.tile([P, T], dt32, tag="psB")
                            pss[m] = ps_m
                        for kc in range(KC):
                            for m in ms:
                                for cg in range(NCH):
                                    nc.tensor.matmul(
                                        pss[m][:, cg * W:(cg + 1) * W],
                                        w_tiles[kc][:, m * P:(m + 1) * P],
                                        xts[kc][cg][:],
                                        start=(kc == 0), stop=(kc == KC - 1),
                                        skip_group_check=True)
                        for m in ms:
                            write_out(m, pss[m])

                def write_qT(m, ps):
                    nc.scalar.activation(
                        qT[:, m, :], ps[:],
                        ActF.Identity, bias=bq_sb[:, m:m + 1], scale=1.0)

                def write_kT(m, ps):
                    # unpacked per-head, sibling rows stay zero
                    nc.scalar.activation(
                        kT[0:D, 2 * m, :], ps[0:D, :],
                        ActF.Identity, bias=bk_sb[0:D, m:m + 1], scale=1.0)
                    nc.scalar.activation(
                        kT[D:P, 2 * m + 1, :], ps[D:P, :],
                        ActF.Identity, bias=bk_sb[D:P, m:m + 1], scale=1.0)

                def load_w(w_d):
                    view = w_d.ap().rearrange("(kc p) n -> kc p n", p=P)
                    tiles = []
                    for kc in range(KC):
                        wt = wpool.tile([P, CO], bf16, tag="w")
                        [nc.sync, nc.scalar, nc.gpsimd][kc % 3].dma_start(
                            wt[:], view[kc])
                        tiles.append(wt)
                    return tiles

                qk_stage(wq_t, bq_sb, write_qT)
                qk_stage(load_w(wk_d), bk_sb, write_kT)

                wv_t = load_w(wv_d)
                bvb_v = bvb_sb[:].rearrange("p (h d) -> p h d", d=D)
                for tt in range(TT):
                    ps = psB.tile([P, CO], dt32, tag="psB")
                    for kc in range(KC):
                        nc.tensor.matmul(
                            ps[:],
                            xts[kc][tt * P // W][:, (tt * P % W):(tt * P % W) + P],
                            wv_t[kc][:],
                            start=(kc == 0), stop=(kc == KC - 1))
                    nc.vector.scalar_tensor_tensor(
                        out=vaug[:, tt, :, 0:D],
                        in0=ps[:].rearrange("p (h d) -> p h d", d=D),
                        scalar=1.0, in1=bvb_v,
                        op0=Alu.mult, op1=Alu.add)

            # ---------------- stage C: attention per head ----------------
            late_cm = tc.tile_pool(name="late", bufs=1)
            late = late_cm.__enter__()
            yT = late.tile([P, NP, T], f32r, tag="yT")
            with nc.named_scope("attn"), \
                 tc.tile_pool(name="ptp", bufs=3) as ptpool, \
                 tc.tile_pool(name="nrm", bufs=1) as nrmpool, \
                 tc.tile_pool(name="ysp", bufs=2) as yspool, \
                 tc.tile_pool(name="psS", bufs=2, space="PSUM") as psS, \
                 tc.tile_pool(name="psY", bufs=1, space="PSUM") as psY:
                yts = {}

                def emit_s(h, j):
                    m = h // 2
                    jb = j * P
                    span = T - jb
                    pt = ptpool.tile([P, span], bf16, tag="pt")
                    for sc in range(jb // WS, T // WS):
                        qs0 = max(WS * sc, jb)
                        sps = psS.tile([P, WS], dt32, tag="s")
                        for cg in range(qs0 // W, (WS * (sc + 1)) // W):
                            qs = max(W * cg, qs0)
                            w = W * (cg + 1) - qs
                            nc.tensor.matmul(
                                sps[:, qs - WS * sc:qs - WS * sc + w],
                                kT[:, h, jb:jb + P],
                                qT[:, m, qs:qs + w],
                                start=True, stop=True,
                                skip_group_check=True)
                        nc.scalar.activation(
                            pt[:, qs0 - jb:WS * (sc + 1) - jb],
                            sps[:, qs0 - WS * sc:WS],
                            ActF.Exp, scale=scale)
                    nc.vector.tensor_mul(pt[:, 0:P], pt[:, 0:P], tri[:])
                    return pt

                def emit_pv(h, j, pt):
                    jb = j * P
                    if j == 0:
                        yt_new = psY.tile([MV, T], dt32, tag="yt")
                        yts[h] = yt_new
                    yt = yts[h]
                    for cg in range(jb // W, NCH):
                        qs = max(W * cg, jb)
                        w = W * (cg + 1) - qs
                        last_j = (W * (cg + 1)) // P - 1
                        nc.tensor.matmul(
                            yt[:, qs:qs + w],
                            vaug[:, j, h, :],
                            pt[:, qs - jb:qs - jb + w],
                            start=(j == 0), stop=(j == last_j),
                            skip_group_check=True)

                T2 = T // 2

                def finish_half(h, hf):
                    # copy PSUM accumulator half out quickly (releases it for
                    # the next head); the slow normalize chain runs SBUF-side.
                    m, r0 = h // 2, (h % 2) * D
                    yt = yts[h]
                    lo = hf * T2
                    ys = yspool.tile([D + 1, T2], dt32, tag="ys")
                    nc.vector.tensor_copy(ys[:], yt[0:D + 1, lo:lo + T2])
                    nc.sync.dma_start(
                        lsc_d.ap()[lo:lo + T2].rearrange("(o t) -> o t", o=1),
                        ys[D:D + 1, :])
                    l128 = nrmpool.tile([P, T2 // P], dt32, tag="l128")
                    nc.gpsimd.dma_start(
                        l128[:],
                        lsc_d.ap()[lo:lo + T2].rearrange("(p c) -> p c", p=P))
                    nc.vector.reciprocal(l128[:], l128[:])
                    nc.gpsimd.dma_start(
                        lsc2_d.ap()[lo:lo + T2].rearrange("(p c) -> p c", p=P),
                        l128[:])
                    bc = nrmpool.tile([D, T2], dt32, tag="bc")
                    nc.sync.dma_start(
                        bc[:],
                        lsc2_d.ap()[lo:lo + T2].rearrange(
                            "(o t) -> o t", o=1).broadcast_to([D, T2]))
                    nc.vector.tensor_mul(
                        yT[r0:r0 + D, m, lo:lo + T2], ys[0:D, :], bc[:])

                def finish_head(h):
                    finish_half(h, 1)
                    yts.pop(h)

                pending = []
                for h in range(HC):
                    for j in range(TT):
                        pt = emit_s(h, j)
                        pending.append((h, j, pt))
                        if len(pending) > 2:
                            ph, pj, ppt = pending.pop(0)
                            emit_pv(ph, pj, ppt)
                            if pj == T // 2 // P - 1:
                                finish_half(ph, 0)
                            if pj == TT - 1:
                                finish_head(ph)
                for ph, pj, ppt in pending:
                    emit_pv(ph, pj, ppt)
                    if pj == T // 2 // P - 1:
                        finish_half(ph, 0)
                    if pj == TT - 1:
                        finish_head(ph)

            # ---------------- stage E: output projection ----------------
            with nc.named_scope("proj"), \
                 tc.tile_pool(name="wpp", bufs=1) as wppool, \
                 tc.tile_pool(name="ost", bufs=3) as opool, \
                 tc.tile_pool(name="psO", bufs=4, space="PSUM") as psO:
                wpsb = wppool.tile([P, NP, C], f32r)
                nc.sync.dma_start(
                    wpsb[:], wp_d.ap().rearrange("(kt p) n -> p kt n", p=P))
                for tt in range(TT):
                    po = psO.tile([P, C], dt32, tag="o")
                    for kt in range(NP):
                        for nn in range(NW):
                            nc.tensor.matmul(
                                po[:, nn * WO:(nn + 1) * WO],
                                yT[:, kt, tt * P:(tt + 1) * P],
                                wpsb[:, kt, nn * WO:(nn + 1) * WO],
                                start=(kt == 0), stop=(kt == NP - 1),
                                skip_group_check=True)
                    ot = opool.tile([P, C], dt32, tag="ot")
                    nc.scalar.activation(ot[:], po[:], ActF.Copy)
                    [nc.sync, nc.scalar, nc.gpsimd][tt % 3].dma_start(
                        out_d.ap()[tt * P:(tt + 1) * P, :], ot[:])
            late_cm.__exit__(None, None, None)

    nc.compile()
    return nc


def make_core_inputs(x, W_attn, b_attn, W_proj, n_cores=8, HC=8, D=64):
    """Host-side sharding: per-core input dicts."""
    B, T, C = x.shape
    CO = HC * D
    NP = CO // P
    in_maps = []
    for c in range(n_cores):
        b = c // (n_cores // B)
        h0 = (c % (n_cores // B)) * HC
        lo = h0 * D
        bq = b_attn[lo:lo + CO]
        bk = b_attn[C + lo:C + lo + CO]
        bv = b_attn[2 * C + lo:2 * C + lo + CO]
        bf = _bf16_np()
        in_maps.append({
            "xt": np.ascontiguousarray(x[b].T).astype(bf),
            "wq": np.ascontiguousarray(W_attn[:, lo:lo + CO]).astype(bf),
            "wk": np.ascontiguousarray(W_attn[:, C + lo:C + lo + CO]).astype(bf),
            "wv": np.ascontiguousarray(W_attn[:, 2 * C + lo:2 * C + lo + CO]).astype(bf),
            "bq": np.ascontiguousarray(bq.reshape(NP, P).T),
            "bk": np.ascontiguousarray(bk.reshape(NP, P).T),
            "bvb": np.tile(bv[None, :], (P, 1)),
            "ones": np.ones((P, (T // P) * HC), _bf16_np()),
            "wp": np.ascontiguousarray(W_proj[lo:lo + CO, :]),
        })
    return in_maps


_CACHE = {}


def _get_program():
    if "nc" not in _CACHE:
        _CACHE["nc"] = build_program()
    return _CACHE["nc"]


def run_on_cores(x, W_attn, b_attn, W_proj, b_proj, trace=False):
    """Returns (full output [B,T,C], BassKernelResults)."""
    from concourse.bass_utils import run_bass_kernel_spmd

    x = np.asarray(x, np.float32)
    W_attn = np.asarray(W_attn, np.float32)
    b_attn = np.asarray(b_attn, np.float32)
    W_proj = np.asarray(W_proj, np.float32)
    b_proj = np.asarray(b_proj, np.float32)

    nc = _get_program()
    in_maps = make_core_inputs(x, W_attn, b_attn, W_proj)
    res = run_bass_kernel_spmd(nc, in_maps, core_ids=list(range(8)), trace=trace)
    B, T, C = x.shape
    out = np.empty((B, T, C), np.float32)
    for b in range(B):
        out[b] = (res.results[2 * b]["out"] + res.results[2 * b + 1]["out"]
                  + b_proj[None, :])
    return out, res


def kernel(x, W_attn, b_attn, W_proj, b_proj):
    out, _ = run_on_cores(x, W_attn, b_attn, W_proj, b_proj, trace=False)
    return out


# revision 40
# speedup vs baseline: 1.8321x; 1.0565x over previous
"""Causal self-attention (B=4, T=2048, C=1024, H=16) on 8 trn2 NeuronCores.

Sharding: core c -> batch b = c//2, heads h0 = (c%2)*8 .. h0+8 (tensor
parallel over heads: c_attn columns / c_proj rows split). Each core computes a
partial projection output [T, C]; the host sums the two partials per batch and
adds b_proj.

Device-side dataflow (all matmuls in float32r = full PE rate, fp32 data):
  - host passes x[b] pre-transposed as xt [C, T]
  - qT, kT  [C_head, T] computed with W_attn column-slices as stationary
  - v computed in natural [T, D] layout, augmented with a ones column so the
    PV matmul also produces the softmax denominator (row 64 of yT_aug)
  - S^T tiles [Tk=128, Tq<=512] = kT_tile^T . qT_chunk  (causal: only the
    lower triangle of S, i.e. Tq >= Tk tiles, is computed)
  - P~ = exp(S^T * 0.125) on ScalarE (no max-subtraction: scores are O(1));
    diagonal 128x128 blocks masked with an upper-triangular 0/1 mask
  - yT_aug [65, T] += v_aug_tile^T . P~  accumulated in PSUM over k-tiles
  - normalize: reciprocal of row 64, gpsimd partition-broadcast, DVE multiply
  - proj: out_tile [128, C] = yT_tile^T . W_proj_rows, streamed to DRAM
"""

import numpy as np

P = 128


def _bf16_np():
    import ml_dtypes
    return ml_dtypes.bfloat16


def build_program(T=2048, C=1024, HC=8, D=64, num_devices=8, trn="TRN2"):
    import concourse.mybir as mybir
    import concourse.tile as tile
    from concourse import bacc
    from concourse.masks import make_upper_triangular

    W = min(512, T)  # matmul moving-dim chunk
    WS = min(1024, T)  # score-PSUM superchunk (exp granularity)
    KC = C // P      # contraction tiles over C
    CO = HC * D      # this core's qkv channel block (512)
    NP = CO // P     # head pairs (2 heads of 64 = 1 partition tile)
    TT = T // P      # T tiles
    NCH = T // W     # T chunks
    WO = min(512, C)  # proj output column chunk
    NW = C // WO     # output column chunks
    dt32 = mybir.dt.float32
    f32r = mybir.dt.float32r
    bf16 = mybir.dt.bfloat16
    ActF = mybir.ActivationFunctionType
    Alu = mybir.AluOpType
    scale = 1.0 / float(np.sqrt(D))

    nc = bacc.Bacc(trn, target_bir_lowering=False, debug=False,
                   enable_asserts=False, num_devices=num_devices)

    xt_d = nc.dram_tensor("xt", [C, T], bf16, kind="ExternalInput")
    wq_d = nc.dram_tensor("wq", [C, CO], bf16, kind="ExternalInput")
    wk_d = nc.dram_tensor("wk", [C, CO], bf16, kind="ExternalInput")
    wv_d = nc.dram_tensor("wv", [C, CO], bf16, kind="ExternalInput")
    bq_d = nc.dram_tensor("bq", [P, NP], dt32, kind="ExternalInput")
    bk_d = nc.dram_tensor("bk", [P, NP], dt32, kind="ExternalInput")
    bvb_d = nc.dram_tensor("bvb", [P, CO], dt32, kind="ExternalInput")
    ones_d = nc.dram_tensor("ones", [P, TT * HC], bf16, kind="ExternalInput")
    wp_d = nc.dram_tensor("wp", [CO, C], f32r, kind="ExternalInput")
    out_d = nc.dram_tensor("out", [T, C], dt32, kind="ExternalOutput")
    lsc_d = nc.dram_tensor("lsc", [T], dt32)
    lsc2_d = nc.dram_tensor("lsc2", [T], dt32)

    with tile.TileContext(nc) as tc:
        with tc.tile_pool(name="const", bufs=1) as cpool, \
             tc.tile_pool(name="pers", bufs=1) as pers:
            tri = cpool.tile([P, P], bf16)
            make_upper_triangular(nc, tri[:], val=1.0, diag=True)
            bq_sb = cpool.tile([P, NP], dt32)
            nc.sync.dma_start(bq_sb[:], bq_d.ap())
            bk_sb = cpool.tile([P, NP], dt32)
            nc.sync.dma_start(bk_sb[:], bk_d.ap())
            bvb_sb = cpool.tile([P, CO], dt32)
            nc.sync.dma_start(bvb_sb[:], bvb_d.ap())

            MV = 96  # PV stationary columns (3 full 32-col PE groups)
            qT = pers.tile([P, NP, T], bf16, tag="qT")
            kT = pers.tile([P, HC, T], bf16, tag="kT")
            vaug = pers.tile([P, TT, HC, MV], bf16, tag="vaug")
            nc.vector.memset(kT[:], 0.0)
            nc.vector.memset(vaug[:], 0.0)
            nc.sync.dma_start(
                vaug[:, :, :, D],
                ones_d.ap().rearrange("p (a b) -> p a b", b=HC))

            # ---------------- stage B: qkv projections ----------------
            with nc.named_scope("qkv"), \
                 tc.tile_pool(name="xtp", bufs=KC * NCH) as xpool, \
                 tc.tile_pool(name="wp_in", bufs=KC) as wpool, \
                 tc.tile_pool(name="psB", bufs=2, space="PSUM") as psB:
                xt_view = xt_d.ap().rearrange("(kc p) t -> kc p t", p=P)
                wq_view = wq_d.ap().rearrange("(kc p) n -> kc p n", p=P)
                xts = []
                wq_t = []
                dmae = [nc.sync, nc.scalar, nc.gpsimd]
                di = 0
                for kc in range(KC):
                    wt = wpool.tile([P, CO], bf16, tag="w")
                    dmae[di % 3].dma_start(wt[:], wq_view[kc])
                    di += 1
                    wq_t.append(wt)
                    row = []
                    for cg in range(NCH):
                        xtc = xpool.tile([P, W], bf16, tag="xt")
                        dmae[di % 3].dma_start(
                            xtc[:], xt_view[kc][:, cg * W:(cg + 1) * W])
                        di += 1
                        row.append(xtc)
                    xts.append(row)

                def qk_stage(w_tiles, bias_sb, write_out):
                    for half in range((NP + 1) // 2):
                        ms = [m for m in (2 * half, 2 * half + 1) if m < NP]
                        pss = {}
                        for m in ms:
                            ps_m = psB.tile([P, T], dt32, tag="psB")
                            pss[m] = ps_m
                        for kc in range(KC):
                            for m in ms:
                                for cg in range(NCH):
                                    nc.tensor.matmul(
                                        pss[m][:, cg * W:(cg + 1) * W],
                                        w_tiles[kc][:, m * P:(m + 1) * P],
                                        xts[kc][cg][:],
                                        start=(kc == 0), stop=(kc == KC - 1),
                                        skip_group_check=True)
                        for m in ms:
                            write_out(m, pss[m])

                def write_qT(m, ps):
                    nc.scalar.activation(
                        qT[:, m, :], ps[:],
                        ActF.Identity, bias=bq_sb[:, m:m + 1], scale=1.0)

                def write_kT(m, ps):
                    # unpacked per-head, sibling rows stay zero
                    nc.scalar.activation(
                        kT[0:D, 2 * m, :], ps[0:D, :],
                        ActF.Identity, bias=bk_sb[0:D, m:m + 1], scale=1.0)
                    nc.scalar.activation(
                        kT[D:P, 2 * m + 1, :], ps[D:P, :],
                        ActF.Identity, bias=bk_sb[D:P, m:m + 1], scale=1.0)

                def load_w(w_d):
                    view = w_d.ap().rearrange("(kc p) n -> kc p n", p=P)
                    tiles = []
                    for kc in range(KC):
                        wt = wpool.tile([P, CO], bf16, tag="w")
                        [nc.sync, nc.scalar, nc.gpsimd][kc % 3].dma_start(
                            wt[:], view[kc])
                        tiles.append(wt)
                    return tiles

                qk_stage(wq_t, bq_sb, write_qT)
                qk_stage(load_w(wk_d), bk_sb, write_kT)

                wv_t = load_w(wv_d)
                bvb_v = bvb_sb[:].rearrange("p (h d) -> p h d", d=D)
                for tt in range(TT):
                    ps = psB.tile([P, CO], dt32, tag="psB")
                    for kc in range(KC):
                        nc.tensor.matmul(
                            ps[:],
                            xts[kc][tt * P // W][:, (tt * P % W):(tt * P % W) + P],
                            wv_t[kc][:],
                            start=(kc == 0), stop=(kc == KC - 1))
                    nc.vector.scalar_tensor_tensor(
                        out=vaug[:, tt, :, 0:D],
                        in0=ps[:].rearrange("p (h d) -> p h d", d=D),
                        scalar=1.0, in1=bvb_v,
                        op0=Alu.mult, op1=Alu.add)

            # ---------------- stage C: attention per head ----------------
            late_cm = tc.tile_pool(name="late", bufs=1)
            late = late_cm.__enter__()
            yT = late.tile([P, NP, T], f32r, tag="yT")
            with nc.named_scope("attn"), \
                 tc.tile_pool(name="ptp", bufs=4) as ptpool, \
                 tc.tile_pool(name="nrm", bufs=1) as nrmpool, \
                 tc.tile_pool(name="ysp", bufs=2) as yspool, \
                 tc.tile_pool(name="psS", bufs=2, space="PSUM") as psS, \
                 tc.tile_pool(name="psY", bufs=1, space="PSUM") as psY:
                yts = {}

                def emit_s(h, j):
                    m = h // 2
                    jb = j * P
                    span = T - jb
                    pt = ptpool.tile([P, span], bf16, tag="pt")
                    for sc in range(jb // WS, T // WS):
                        qs0 = max(WS * sc, jb)
                        sps = psS.tile([P, WS], dt32, tag="s")
                        for cg in range(qs0 // W, (WS * (sc + 1)) // W):
                            qs = max(W * cg, qs0)
                            w = W * (cg + 1) - qs
                            nc.tensor.matmul(
                                sps[:, qs - WS * sc:qs - WS * sc + w],
                                kT[:, h, jb:jb + P],
                                qT[:, m, qs:qs + w],
                                start=True, stop=True,
                                skip_group_check=True)
                        nc.scalar.activation(
                            pt[:, qs0 - jb:WS * (sc + 1) - jb],
                            sps[:, qs0 - WS * sc:WS],
                            ActF.Exp, scale=scale)
                    nc.vector.tensor_mul(pt[:, 0:P], pt[:, 0:P], tri[:])
                    return pt

                def emit_pv(h, j, pt):
                    jb = j * P
                    if j == 0:
                        yt_new = psY.tile([MV, T], dt32, tag="yt")
                        yts[h] = yt_new
                    yt = yts[h]
                    for cg in range(jb // W, NCH):
                        qs = max(W * cg, jb)
                        w = W * (cg + 1) - qs
                        last_j = (W * (cg + 1)) // P - 1
                        nc.tensor.matmul(
                            yt[:, qs:qs + w],
                            vaug[:, j, h, :],
                            pt[:, qs - jb:qs - jb + w],
                            start=(j == 0), stop=(j == last_j),
                            skip_group_check=True)

                T2 = T // 2

                def finish_half(h, hf):
                    # copy PSUM accumulator half out quickly (releases it for
                    # the next head); the slow normalize chain runs SBUF-side.
                    m, r0 = h // 2, (h % 2) * D
                    yt = yts[h]
                    lo = hf * T2
                    ys = yspool.tile([D + 1, T2], dt32, tag="ys")
                    nc.vector.tensor_copy(ys[:], yt[0:D + 1, lo:lo + T2])
                    nc.sync.dma_start(
                        lsc_d.ap()[lo:lo + T2].rearrange("(o t) -> o t", o=1),
                        ys[D:D + 1, :])
                    l128 = nrmpool.tile([P, T2 // P], dt32, tag="l128")
                    nc.gpsimd.dma_start(
                        l128[:],
                        lsc_d.ap()[lo:lo + T2].rearrange("(p c) -> p c", p=P))
                    nc.vector.reciprocal(l128[:], l128[:])
                    nc.gpsimd.dma_start(
                        lsc2_d.ap()[lo:lo + T2].rearrange("(p c) -> p c", p=P),
                        l128[:])
                    bc = nrmpool.tile([D, T2], dt32, tag="bc")
                    nc.sync.dma_start(
                        bc[:],
                        lsc2_d.ap()[lo:lo + T2].rearrange(
                            "(o t) -> o t", o=1).broadcast_to([D, T2]))
                    nc.vector.tensor_mul(
                        yT[r0:r0 + D, m, lo:lo + T2], ys[0:D, :], bc[:])

                def finish_head(h):
                    finish_half(h, 1)
                    yts.pop(h)

                pending = []
                for h in range(HC):
                    for j in range(TT):
                        pt = emit_s(h, j)
                        pending.append((h, j, pt))
                        if len(pending) > 3:
                            ph, pj, ppt = pending.pop(0)
                            emit_pv(ph, pj, ppt)
                            if pj == T // 2 // P - 1:
                                finish_half(ph, 0)
                            if pj == TT - 1:
                                finish_head(ph)
                for ph, pj, ppt in pending:
                    emit_pv(ph, pj, ppt)
                    if pj == T // 2 // P - 1:
                        finish_half(ph, 0)
                    if pj == TT - 1:
                        finish_head(ph)

            # ---------------- stage E: output projection ----------------
            with nc.named_scope("proj"), \
                 tc.tile_pool(name="wpp", bufs=1) as wppool, \
                 tc.tile_pool(name="ost", bufs=3) as opool, \
                 tc.tile_pool(name="psO", bufs=4, space="PSUM") as psO:
                wpsb = wppool.tile([P, NP, C], f32r)
                nc.sync.dma_start(
                    wpsb[:], wp_d.ap().rearrange("(kt p) n -> p kt n", p=P))
                for tt in range(TT):
                    po = psO.tile([P, C], dt32, tag="o")
                    for kt in range(NP):
                        for nn in range(NW):
                            nc.tensor.matmul(
                                po[:, nn * WO:(nn + 1) * WO],
                                yT[:, kt, tt * P:(tt + 1) * P],
                                wpsb[:, kt, nn * WO:(nn + 1) * WO],
                                start=(kt == 0), stop=(kt == NP - 1),
                                skip_group_check=True)
                    ot = opool.tile([P, C], dt32, tag="ot")
                    nc.scalar.activation(ot[:], po[:], ActF.Copy)
                    [nc.sync, nc.scalar, nc.gpsimd][tt % 3].dma_start(
                        out_d.ap()[tt * P:(tt + 1) * P, :], ot[:])
            late_cm.__exit__(None, None, None)

    nc.compile()
    return nc


def make_core_inputs(x, W_attn, b_attn, W_proj, n_cores=8, HC=8, D=64):
    """Host-side sharding: per-core input dicts."""
    B, T, C = x.shape
    CO = HC * D
    NP = CO // P
    in_maps = []
    for c in range(n_cores):
        b = c // (n_cores // B)
        h0 = (c % (n_cores // B)) * HC
        lo = h0 * D
        bq = b_attn[lo:lo + CO]
        bk = b_attn[C + lo:C + lo + CO]
        bv = b_attn[2 * C + lo:2 * C + lo + CO]
        bf = _bf16_np()
        in_maps.append({
            "xt": np.ascontiguousarray(x[b].T).astype(bf),
            "wq": np.ascontiguousarray(W_attn[:, lo:lo + CO]).astype(bf),
            "wk": np.ascontiguousarray(W_attn[:, C + lo:C + lo + CO]).astype(bf),
            "wv": np.ascontiguousarray(W_attn[:, 2 * C + lo:2 * C + lo + CO]).astype(bf),
            "bq": np.ascontiguousarray(bq.reshape(NP, P).T),
            "bk": np.ascontiguousarray(bk.reshape(NP, P).T),
            "bvb": np.tile(bv[None, :], (P, 1)),
            "ones": np.ones((P, (T // P) * HC), _bf16_np()),
            "wp": np.ascontiguousarray(W_proj[lo:lo + CO, :]),
        })
    return in_maps


_CACHE = {}


def _get_program():
    if "nc" not in _CACHE:
        _CACHE["nc"] = build_program()
    return _CACHE["nc"]


def run_on_cores(x, W_attn, b_attn, W_proj, b_proj, trace=False):
    """Returns (full output [B,T,C], BassKernelResults)."""
    from concourse.bass_utils import run_bass_kernel_spmd

    x = np.asarray(x, np.float32)
    W_attn = np.asarray(W_attn, np.float32)
    b_attn = np.asarray(b_attn, np.float32)
    W_proj = np.asarray(W_proj, np.float32)
    b_proj = np.asarray(b_proj, np.float32)

    nc = _get_program()
    in_maps = make_core_inputs(x, W_attn, b_attn, W_proj)
    res = run_bass_kernel_spmd(nc, in_maps, core_ids=list(range(8)), trace=trace)
    B, T, C = x.shape
    out = np.empty((B, T, C), np.float32)
    for b in range(B):
        out[b] = (res.results[2 * b]["out"] + res.results[2 * b + 1]["out"]
                  + b_proj[None, :])
    return out, res


def kernel(x, W_attn, b_attn, W_proj, b_proj):
    out, _ = run_on_cores(x, W_attn, b_attn, W_proj, b_proj, trace=False)
    return out
